# revision 1
# baseline (speedup 1.0000x reference)
"""Trainium2 Bass kernel v2 for the 2-layer multi-head GAT.

Design (8 NeuronCores, SPMD):
  - Gather tables: G1[v] = [h(v) (d,h)-interleaved (512) | s_tgt (8) | pad]
    as 640 bf16 = 1280B rows; G2[v] = [h2 fp8 (128) | s_tgt2 bf16 | s_src2
    bf16 | pad] as 256 fp8-bytes = 256B rows (fp8 payload halves the
    AllGather).
  - Edge gathers via dma_gather (Ant SWDGE custom op): int16 indices force a
    split of the node space into <=32768-row chunks; one dma_gather per
    (node-tile, chunk) pulls that chunk's edge slots (group-padded to 128).
  - Per (tile, chunk): c_k "identity layers" (layer j, lane p = node p's j-th
    chunk-k edge; segment-sum = PSUM accumulate with identity stationary) +
    m_k masked overflow groups (maskT matmul).
  - Pad slots gather chunk row 0 and carry a host-crafted edge-attr column
    v_pad whose (ae1|ae2)-projections are -1e4, so exp(lrelu(score))
    underflows to 0 in bf16 and pads contribute exactly nothing.
  - Scores batched per tile; exp on ScalarE; attention normalized BEFORE the
    message multiply (exn = ex * rcp[src]); message mult at DVE 2x bf16 via
    the (d,h) interleave (broadcast over d keeps the last dim packed).
  - Layer-2 h2 = x1 @ W2X via 4 PE transposes (copies on ScalarE); AllGather
    of the fp8 G2 shard; same edge machinery with 1 head; fused log_softmax.
"""

import numpy as np

import concourse.bass as bass
import concourse.bacc as bacc
import concourse.mybir as mybir
import concourse.tile as tile

F32 = mybir.dt.float32
BF16 = mybir.dt.bfloat16
FP8 = mybir.dt.float8e4
I16 = mybir.dt.int16

N_CORES = 8
P = 128
H = 8
DH = 64
DIN = 128
DC = 512
DOUT = 128
EA = 16
LRELU = 0.01
G1W = 640         # row elems (bf16): [h 512 | s_tgt 8 | pad]
G1B = 520         # written row body
G2W = 256         # row bytes (fp8 dtype): [h2 fp8 128 | stgt2 bf16 | ssrc2]
CHROWS = 32768
EPS0 = 1e-30
PAD_SCORE = -1.0e4


def _gdef(cks, mks):
    """Group definitions: list of (chunk, is_identity, masked_idx)."""
    gdef = []
    km = 0
    for k, (ck, mk) in enumerate(zip(cks, mks)):
        for _ in range(ck):
            gdef.append((k, True, None))
        for _ in range(mk):
            gdef.append((k, False, km))
            km += 1
    return gdef, km


def build_program(NP, cks, mks, repeat=1, dbg=0, phases=4, e1sub=0):
    cks, mks = tuple(cks), tuple(mks)
    GDEF, M = _gdef(cks, mks)
    G = len(GDEF)
    NCH = len(cks)
    cbase = np.cumsum([0] + [cks[k] + mks[k] for k in range(NCH)])
    chbase = [k * CHROWS for k in range(NCH)]
    chsize = [min(CHROWS, NP - k * CHROWS) for k in range(NCH)]
    id_runs = [(int(cbase[k]), cks[k]) for k in range(NCH) if cks[k] > 0]
    mk_runs = []
    km = 0
    for k in range(NCH):
        if mks[k] > 0:
            mk_runs.append((int(cbase[k]) + cks[k], km, mks[k]))
            km += mks[k]

    NT_ALL = NP // P
    NSH = NP // N_CORES
    NT = NSH // P

    nc = bacc.Bacc("TRN2", target_bir_lowering=False, debug=False,
                   num_devices=N_CORES)

    XTT = nc.dram_tensor("xtt", [NT_ALL, DIN, P], BF16, kind="ExternalInput")
    XTO = nc.dram_tensor("xto", [NT, DIN, P], BF16, kind="ExternalInput")
    WC = nc.dram_tensor("wc", [DIN, DC], BF16, kind="ExternalInput")
    WAT = nc.dram_tensor("wat", [DIN, H], BF16, kind="ExternalInput")
    WAS = nc.dram_tensor("was", [DIN, H], BF16, kind="ExternalInput")
    AE1 = nc.dram_tensor("ae1", [EA, H], BF16, kind="ExternalInput")
    AE2 = nc.dram_tensor("ae2", [EA, 1], BF16, kind="ExternalInput")
    W2X = nc.dram_tensor("w2x", [P, 4 * 130], BF16, kind="ExternalInput")
    IOTP = nc.dram_tensor("iotp", [P, 1], BF16, kind="ExternalInput")
    IOTF = nc.dram_tensor("iotf", [P, P], BF16, kind="ExternalInput")
    IDT = nc.dram_tensor("idt", [P, P], BF16, kind="ExternalInput")
    ON1 = nc.dram_tensor("on1", [1, P], BF16, kind="ExternalInput")
    IDX16 = nc.dram_tensor("idx16", [P, NT * G * 8], I16,
                           kind="ExternalInput")
    SRCLM = nc.dram_tensor("srclm", [P, NT * M], BF16, kind="ExternalInput")
    SROWM = nc.dram_tensor("srowm", [NT, 1, M * P], BF16,
                           kind="ExternalInput")
    EATT = nc.dram_tensor("eatt", [EA, NT * G * P], BF16,
                          kind="ExternalInput")

    G1 = nc.dram_tensor("g1", [NP, G1W], BF16)
    G2S = nc.dram_tensor("g2s", [NSH, G2W], FP8)
    G2F = nc.dram_tensor("g2f", [NP, G2W], FP8, addr_space="Shared")

    OUT = nc.dram_tensor("out", [NSH, DOUT], F32, kind="ExternalOutput")
    if dbg:
        G1D = nc.dram_tensor("g1d", [dbg, G1W], BF16, kind="ExternalOutput")
        G2SD = nc.dram_tensor("g2sd", [NSH, G2W], FP8, kind="ExternalOutput")

    AX = mybir.AxisListType.X
    OP = mybir.AluOpType
    AF = mybir.ActivationFunctionType

    with tile.TileContext(nc) as tc, \
         tc.tile_pool(name="const", bufs=1) as cp:
        wc_sb = cp.tile([DIN, DC], BF16, tag="wc")
        nc.sync.dma_start(out=wc_sb[:], in_=WC[:])
        wat_sb = cp.tile([DIN, H], BF16, tag="wat")
        nc.sync.dma_start(out=wat_sb[:], in_=WAT[:])
        was_sb = cp.tile([DIN, H], BF16, tag="was")
        nc.sync.dma_start(out=was_sb[:], in_=WAS[:])
        ae1_sb = cp.tile([EA, H], BF16, tag="ae1")
        nc.sync.dma_start(out=ae1_sb[:], in_=AE1[:])
        ae2_sb = cp.tile([EA, 1], BF16, tag="ae2")
        nc.sync.dma_start(out=ae2_sb[:], in_=AE2[:])
        w2x_sb = cp.tile([P, 4 * 130], BF16, tag="w2x")
        nc.sync.dma_start(out=w2x_sb[:], in_=W2X[:])
        iotp_sb = cp.tile([P, 1], BF16, tag="iotp")
        nc.sync.dma_start(out=iotp_sb[:], in_=IOTP[:])
        iotf_sb = cp.tile([P, P], BF16, tag="iotf")
        nc.sync.dma_start(out=iotf_sb[:], in_=IOTF[:])
        idt_sb = cp.tile([P, P], BF16, tag="idt")
        nc.sync.dma_start(out=idt_sb[:], in_=IDT[:])
        on1_sb = cp.tile([1, P], BF16, tag="on1")
        nc.sync.dma_start(out=on1_sb[:], in_=ON1[:])
        idx_sb = cp.tile([P, NT * G * 8], I16, tag="idx16")
        nc.sync.dma_start(out=idx_sb[:], in_=IDX16[:])
        srclm_sb = cp.tile([P, NT * M], BF16, tag="srclm")
        nc.sync.dma_start(out=srclm_sb[:], in_=SRCLM[:])
        s2all_sb = cp.tile([P, NT], BF16, tag="s2all")
        neg1_sb = cp.tile([P, 1], F32, tag="neg1")
        nc.vector.memset(neg1_sb[:], -1.0)

        for _rep in range(repeat):
            if _rep:
                tc.strict_bb_all_engine_barrier()

            # ---------------- Phase D1: build G1 table (replicated) --------
            with tc.tile_pool(name="d1", bufs=4) as dp, \
                 tc.tile_pool(name="d1ps", bufs=4, space="PSUM") as dps:
                B4 = 4
                for ib in range(0, NT_ALL, B4):
                    nb = min(B4, NT_ALL - ib)
                    xt = dp.tile([P, B4 * DIN], BF16, tag="xt")
                    nc.sync.dma_start(
                        out=xt[:].rearrange("p (b k) -> p b k", b=B4)[:, 0:nb],
                        in_=XTT[ib:ib + nb].rearrange("b k p -> k b p"))
                    g1t = dp.tile([P, B4 * G1B], BF16, tag="g1t")
                    for k in range(nb):
                        i = ib + k
                        ph = dps.tile([P, DC], F32, tag="ph")
                        nc.tensor.matmul(ph[:], xt[:, k * DIN:(k + 1) * DIN],
                                         wc_sb[:], start=True, stop=True)
                        ps = dps.tile([P, H], F32, tag="ps")
                        nc.tensor.matmul(ps[:], xt[:, k * DIN:(k + 1) * DIN],
                                         wat_sb[:], start=True, stop=True)
                        gs = g1t[:, k * G1B:(k + 1) * G1B]
                        eng = (nc.vector, nc.scalar)[i % 2]
                        if eng is nc.scalar:
                            nc.scalar.activation(gs[:, 0:DC], ph[:], AF.Copy)
                            nc.scalar.activation(gs[:, DC:G1B], ps[:],
                                                 AF.Copy)
                        else:
                            eng.tensor_copy(out=gs[:, 0:DC], in_=ph[:])
                            eng.tensor_copy(out=gs[:, DC:G1B], in_=ps[:])
                    nc.sync.dma_start(
                        out=G1[ib * P:(ib + nb) * P, 0:G1B]
                            .rearrange("(b p) w -> p b w", p=P),
                        in_=g1t[:].rearrange("p (b w) -> p b w", b=B4)
                            [:, 0:nb])

            tc.strict_bb_all_engine_barrier()

            if dbg:
                with tc.tile_pool(name="dbg1", bufs=2) as dbp:
                    for i in range(dbg // P):
                        t = dbp.tile([P, G1W], BF16, tag="t")
                        nc.sync.dma_start(out=t[:],
                                          in_=G1[i * P:(i + 1) * P, :])
                        nc.sync.dma_start(out=G1D[i * P:(i + 1) * P, :],
                                          in_=t[:])
                tc.strict_bb_all_engine_barrier()

            # ---------------- Phase E1: layer-1 edge pass ------------------
            if phases < 2:
                continue
            with tc.tile_pool(name="e1g", bufs=3) as gp, \
                 tc.tile_pool(name="e1w", bufs=2) as ep, \
                 tc.tile_pool(name="e1s", bufs=2) as sp, \
                 tc.tile_pool(name="e1r", bufs=4) as rp, \
                 tc.tile_pool(name="e1x", bufs=2) as xp, \
                 tc.tile_pool(name="psA", bufs=2, space="PSUM") as psA, \
                 tc.tile_pool(name="psB", bufs=1, space="PSUM") as psB, \
                 tc.tile_pool(name="psM", bufs=2, space="PSUM") as psM, \
                 tc.tile_pool(name="psT", bufs=2, space="PSUM") as psT, \
                 tc.tile_pool(name="psG", bufs=1, space="PSUM") as psG:
                for nt in range(NT):
                    gdst = gp.tile([P, G * G1W], BF16, tag="g")
                    for k in range(NCH):
                        gk = cks[k] + mks[k]
                        cb = int(cbase[k])
                        off = 0
                        while off < gk:
                            gn = min(8, gk - off)
                            c0 = cb + off
                            nc.gpsimd.dma_gather(
                                gdst[:, c0 * G1W:(c0 + gn) * G1W]
                                    .rearrange("p (g w) -> p g w", w=G1W),
                                G1[chbase[k]:chbase[k] + chsize[k], :],
                                idx_sb[:, (nt * G + c0) * 8:
                                       (nt * G + c0 + gn) * 8],
                                gn * P,
                                gn * P,
                                G1W,
                            )
                            off += gn
                    eat = ep.tile([EA, G * P], BF16, tag="eat")
                    nc.sync.dma_start(
                        out=eat[:], in_=EATT[:, nt * G * P:(nt + 1) * G * P])
                    xto = ep.tile([P, DIN], BF16, tag="xto")
                    nc.sync.dma_start(out=xto[:], in_=XTO[nt])
                    srow = ep.tile([1, M * P], BF16, tag="srow")
                    nc.sync.dma_start(out=srow[:], in_=SROWM[nt])
                    if e1sub == 1:
                        continue

                    NV = 8 + G * 8 + 8 + M * 8
                    psa = psA.tile([P, NV], F32, tag="a")
                    s1p = psa[:, 0:8]
                    pv = psa[:, 8:8 + G * 8]
                    pd = psa[:, 8 + G * 8:16 + G * 8]
                    prg = psa[:, 16 + G * 8:NV]
                    pbc = psB.tile([P, M * P], F32, tag="b")

                    nc.tensor.matmul(s1p[:], xto[:], was_sb[:],
                                     start=True, stop=True)
                    s1sb = sp.tile([P, H], BF16, tag="s1sb")
                    nc.vector.tensor_copy(out=s1sb[:], in_=s1p[:])

                    nc.tensor.matmul(
                        pbc[:], on1_sb[:], srow[0:1, :],
                        start=True, stop=True)
                    maskv = sp.tile([P, M * P], BF16, tag="maskv")
                    nc.vector.tensor_tensor(
                        out=maskv[:], in0=pbc[:],
                        in1=iotp_sb[:].to_broadcast([P, M * P]),
                        op=OP.is_equal)
                    maskT = sp.tile([P, M * P], BF16, tag="maskT")
                    for km in range(M):
                        nc.vector.tensor_tensor(
                            out=maskT[:, km * P:(km + 1) * P],
                            in0=srclm_sb[:, nt * M + km:nt * M + km + 1]
                                .to_broadcast([P, P]),
                            in1=iotf_sb[:], op=OP.is_equal)

                    for j, (k, is_id, km) in enumerate(GDEF):
                        nc.tensor.matmul(
                            pv[:, j * 8:(j + 1) * 8],
                            eat[:, j * P:(j + 1) * P], ae1_sb[:],
                            start=True, stop=bool(is_id))
                        if not is_id:
                            nc.tensor.matmul(
                                pv[:, j * 8:(j + 1) * 8],
                                maskv[:, km * P:(km + 1) * P], s1sb[:],
                                start=False, stop=True)
                    sc = sp.tile([P, G * 8], BF16, tag="sc")
                    nc.vector.tensor_tensor(
                        out=sc[:].rearrange("p (g h) -> p g h", h=8),
                        in0=pv.rearrange("p (g h) -> p g h", h=8),
                        in1=gdst[:].rearrange("p (g w) -> p g w", w=G1W)
                            [:, :, DC:DC + 8],
                        op=OP.add)
                    for (c0, cn) in id_runs:
                        nc.vector.tensor_tensor(
                            out=sc[:, c0 * 8:(c0 + cn) * 8]
                                .rearrange("p (g h) -> p g h", h=8),
                            in0=sc[:, c0 * 8:(c0 + cn) * 8]
                                .rearrange("p (g h) -> p g h", h=8),
                            in1=s1sb[:].unsqueeze(1).to_broadcast([P, cn, 8]),
                            op=OP.add)
                    lr = sp.tile([P, G * 8], BF16, tag="lr")
                    nc.vector.scalar_tensor_tensor(
                        out=lr[:], in0=sc[:], scalar=LRELU, in1=sc[:],
                        op0=OP.mult, op1=OP.max)
                    ex = sp.tile([P, G * 8], BF16, tag="ex")
                    nc.scalar.activation(ex[:], lr[:], AF.Exp)
                    if e1sub == 2:
                        continue

                    for j, (k, is_id, km) in enumerate(GDEF):
                        lhs = idt_sb[:] if is_id else \
                            maskT[:, km * P:(km + 1) * P]
                        nc.tensor.matmul(pd[:], lhs, ex[:, j * 8:(j + 1) * 8],
                                         start=(j == 0), stop=(j == G - 1))
                    den = sp.tile([P, H], BF16, tag="den")
                    nc.vector.tensor_scalar_add(out=den[:], in0=pd[:],
                                                scalar1=EPS0)
                    rcp = sp.tile([P, H], BF16, tag="rcp")
                    with nc.allow_low_precision(reason="attn rcp bf16 ok"):
                        nc.vector.reciprocal(out=rcp[:], in_=den[:])
                    exn = sp.tile([P, G * 8], BF16, tag="exn")
                    for (c0, cn) in id_runs:
                        nc.vector.tensor_tensor(
                            out=exn[:, c0 * 8:(c0 + cn) * 8]
                                .rearrange("p (g h) -> p g h", h=8),
                            in0=ex[:, c0 * 8:(c0 + cn) * 8]
                                .rearrange("p (g h) -> p g h", h=8),
                            in1=rcp[:].unsqueeze(1).to_broadcast([P, cn, 8]),
                            op=OP.mult)
                    for km in range(M):
                        nc.tensor.matmul(
                            prg[:, km * 8:(km + 1) * 8],
                            maskv[:, km * P:(km + 1) * P], rcp[:],
                            start=(km == 0), stop=(km == M - 1))
                    for (g0, km0, mn) in mk_runs:
                        nc.vector.tensor_tensor(
                            out=exn[:, g0 * 8:(g0 + mn) * 8],
                            in0=ex[:, g0 * 8:(g0 + mn) * 8],
                            in1=prg[:, km0 * 8:(km0 + mn) * 8], op=OP.mult)

                    pm = psM.tile([P, DC], F32, tag="pm")
                    for j, (k, is_id, km) in enumerate(GDEF):
                        rhs = rp.tile([P, DC], BF16, tag="rhs")
                        nc.vector.tensor_tensor(
                            out=rhs[:].rearrange("p (d h) -> p d h", h=8),
                            in0=gdst[:, j * G1W:j * G1W + DC]
                                .rearrange("p (d h) -> p d h", h=8),
                            in1=exn[:, j * 8:(j + 1) * 8].unsqueeze(1)
                                .to_broadcast([P, DH, 8]),
                            op=OP.mult)
                        lhs = idt_sb[:] if is_id else \
                            maskT[:, km * P:(km + 1) * P]
                        nc.tensor.matmul(pm[:], lhs, rhs[:],
                                         start=(j == 0), stop=(j == G - 1))

                    x0 = xp.tile([P, DC], BF16, tag="x0")
                    nc.scalar.activation(x0[:], pm[:], AF.Copy)
                    if e1sub == 3:
                        continue
                    xa = xp.tile([P, DC], BF16, tag="xa")
                    nc.vector.tensor_scalar_min(out=xa[:], in0=x0[:],
                                                scalar1=0.0)
                    xb = xp.tile([P, DC], BF16, tag="xb")
                    nc.scalar.activation(xb[:], xa[:], AF.Exp)
                    xd = xp.tile([P, DC], BF16, tag="xd")
                    nc.scalar.activation(xd[:], xb[:], AF.Exp,
                                         bias=neg1_sb[:])
                    x1 = xp.tile([P, DC], BF16, tag="x1")
                    nc.vector.scalar_tensor_tensor(
                        out=x1[:], in0=xd[:], scalar=-1.0, in1=x0[:],
                        op0=OP.add, op1=OP.max)
                    pg2 = psG.tile([P, 130], F32, tag="pg2")
                    for c4 in range(4):
                        pxT = psT.tile([P, P], BF16, tag="pxT")
                        nc.tensor.transpose(pxT[:],
                                            x1[:, c4 * P:(c4 + 1) * P],
                                            idt_sb[:])
                        xTs = rp.tile([P, P], BF16, tag="xTs")
                        nc.scalar.activation(xTs[:], pxT[:], AF.Copy)
                        nc.tensor.matmul(
                            pg2[:], xTs[:],
                            w2x_sb[:, c4 * 130:(c4 + 1) * 130],
                            start=(c4 == 0), stop=(c4 == 3))
                    g2t = sp.tile([P, G2W], FP8, tag="g2t")
                    nc.vector.tensor_copy(out=g2t[:, 0:DOUT],
                                          in_=pg2[:, 0:DOUT])
                    nc.vector.tensor_copy(
                        out=g2t[:].bitcast(BF16)[:, 64:66],
                        in_=pg2[:, DOUT:DOUT + 2])
                    nc.vector.tensor_copy(out=s2all_sb[:, nt:nt + 1],
                                          in_=pg2[:, 129:130])
                    nc.sync.dma_start(out=G2S[nt * P:(nt + 1) * P, :],
                                      in_=g2t[:])

            tc.strict_bb_all_engine_barrier()

            # ---------------- AllGather G2 shard -> full table -------------
            if phases < 3:
                continue
            with tc.tile_critical():
                with nc.semaphore() as cc_sem:
                    nc.gpsimd.collective_compute(
                        "AllGather", OP.bypass,
                        replica_groups=[list(range(N_CORES))],
                        ins=[G2S[:]], outs=[G2F[0:NP, :]],
                    ).then_inc(cc_sem, 1)
                    nc.gpsimd.wait_ge(cc_sem, 1)

            tc.strict_bb_all_engine_barrier()

            if dbg:
                with tc.tile_pool(name="dbg2", bufs=2) as dbp:
                    for i in range(NSH // P):
                        t = dbp.tile([P, G2W], FP8, tag="t")
                        nc.sync.dma_start(out=t[:],
                                          in_=G2S[i * P:(i + 1) * P, :])
                        nc.sync.dma_start(out=G2SD[i * P:(i + 1) * P, :],
                                          in_=t[:])
                tc.strict_bb_all_engine_barrier()

            # ---------------- Phase E2: layer-2 edge pass + log_softmax ----
            if phases < 4:
                continue
            with tc.tile_pool(name="e2g", bufs=3) as gp, \
                 tc.tile_pool(name="e2w", bufs=2) as ep, \
                 tc.tile_pool(name="e2s", bufs=2) as sp, \
                 tc.tile_pool(name="e2r", bufs=4) as rp, \
                 tc.tile_pool(name="psA2", bufs=2, space="PSUM") as psA, \
                 tc.tile_pool(name="psB2", bufs=1, space="PSUM") as psB, \
                 tc.tile_pool(name="psM2", bufs=2, space="PSUM") as psM:
                for nt in range(NT):
                    gdst = gp.tile([P, G * G2W], FP8, tag="g2")
                    for k in range(NCH):
                        gk = cks[k] + mks[k]
                        cb = int(cbase[k])
                        off = 0
                        while off < gk:
                            gn = min(8, gk - off)
                            c0 = cb + off
                            nc.gpsimd.dma_gather(
                                gdst[:, c0 * G2W:(c0 + gn) * G2W]
                                    .rearrange("p (g w) -> p g w", w=G2W),
                                G2F[chbase[k]:chbase[k] + chsize[k], :],
                                idx_sb[:, (nt * G + c0) * 8:
                                       (nt * G + c0 + gn) * 8],
                                gn * P,
                                gn * P,
                                G2W,
                            )
                            off += gn
                    eat = ep.tile([EA, G * P], BF16, tag="eat")
                    nc.sync.dma_start(
                        out=eat[:], in_=EATT[:, nt * G * P:(nt + 1) * G * P])
                    srow = ep.tile([1, M * P], BF16, tag="srow")
                    nc.sync.dma_start(out=srow[:], in_=SROWM[nt])

                    NV = G + 1 + M
                    psa = psA.tile([P, NV], F32, tag="a")
                    pv = psa[:, 0:G]
                    pd = psa[:, G:G + 1]
                    prg = psa[:, G + 1:NV]
                    pbc = psB.tile([P, M * P], F32, tag="b")

                    nc.tensor.matmul(
                        pbc[:], on1_sb[:], srow[0:1, :],
                        start=True, stop=True)
                    maskv = sp.tile([P, M * P], BF16, tag="maskv")
                    nc.vector.tensor_tensor(
                        out=maskv[:], in0=pbc[:],
                        in1=iotp_sb[:].to_broadcast([P, M * P]),
                        op=OP.is_equal)
                    maskT = sp.tile([P, M * P], BF16, tag="maskT")
                    for km in range(M):
                        nc.vector.tensor_tensor(
                            out=maskT[:, km * P:(km + 1) * P],
                            in0=srclm_sb[:, nt * M + km:nt * M + km + 1]
                                .to_broadcast([P, P]),
                            in1=iotf_sb[:], op=OP.is_equal)

                    for j, (k, is_id, km) in enumerate(GDEF):
                        nc.tensor.matmul(
                            pv[:, j:j + 1], eat[:, j * P:(j + 1) * P],
                            ae2_sb[:], start=True, stop=bool(is_id))
                        if not is_id:
                            nc.tensor.matmul(
                                pv[:, j:j + 1],
                                maskv[:, km * P:(km + 1) * P],
                                s2all_sb[:, nt:nt + 1],
                                start=False, stop=True)
                    sc = sp.tile([P, G], BF16, tag="sc")
                    nc.vector.tensor_tensor(
                        out=sc[:].unsqueeze(2), in0=pv.unsqueeze(2),
                        in1=gdst[:].bitcast(BF16)
                            .rearrange("p (g w) -> p g w", w=G2W // 2)
                            [:, :, 64:65],
                        op=OP.add)
                    for (c0, cn) in id_runs:
                        nc.vector.tensor_tensor(
                            out=sc[:, c0:c0 + cn], in0=sc[:, c0:c0 + cn],
                            in1=s2all_sb[:, nt:nt + 1].to_broadcast([P, cn]),
                            op=OP.add)
                    lr = sp.tile([P, G], BF16, tag="lr")
                    nc.vector.scalar_tensor_tensor(
                        out=lr[:], in0=sc[:], scalar=LRELU, in1=sc[:],
                        op0=OP.mult, op1=OP.max)
                    ex = sp.tile([P, G], BF16, tag="ex")
                    nc.scalar.activation(ex[:], lr[:], AF.Exp)
                    for j, (k, is_id, km) in enumerate(GDEF):
                        lhs = idt_sb[:] if is_id else \
                            maskT[:, km * P:(km + 1) * P]
                        nc.tensor.matmul(pd[:], lhs, ex[:, j:j + 1],
                                         start=(j == 0), stop=(j == G - 1))
                    den = sp.tile([P, 1], BF16, tag="den")
                    nc.vector.tensor_scalar_add(out=den[:], in0=pd[:],
                                                scalar1=EPS0)
                    rcp = sp.tile([P, 1], F32, tag="rcp")
                    with nc.allow_low_precision(reason="attn rcp ok"):
                        nc.vector.reciprocal(out=rcp[:], in_=den[:])
                    rcpb = sp.tile([P, 1], BF16, tag="rcpb")
                    nc.vector.tensor_copy(out=rcpb[:], in_=rcp[:])
                    exn = sp.tile([P, G], F32, tag="exn")
                    for (c0, cn) in id_runs:
                        nc.vector.tensor_tensor(
                            out=exn[:, c0:c0 + cn], in0=ex[:, c0:c0 + cn],
                            in1=rcp[:].to_broadcast([P, cn]), op=OP.mult)
                    for km in range(M):
                        nc.tensor.matmul(
                            prg[:, km:km + 1],
                            maskv[:, km * P:(km + 1) * P], rcpb[:],
                            start=(km == 0), stop=(km == M - 1))
                    for (g0, km0, mn) in mk_runs:
                        nc.vector.tensor_tensor(
                            out=exn[:, g0:g0 + mn], in0=ex[:, g0:g0 + mn],
                            in1=prg[:, km0:km0 + mn], op=OP.mult)

                    pm = psM.tile([P, DOUT], F32, tag="pm")
                    for j, (k, is_id, km) in enumerate(GDEF):
                        rhs = rp.tile([P, DOUT], BF16, tag="rhs")
                        nc.vector.tensor_scalar_mul(
                            out=rhs[:], in0=gdst[:, j * G2W:j * G2W + DOUT],
                            scalar1=exn[:, j:j + 1])
                        lhs = idt_sb[:] if is_id else \
                            maskT[:, km * P:(km + 1) * P]
                        nc.tensor.matmul(pm[:], lhs, rhs[:],
                                         start=(j == 0), stop=(j == G - 1))

                    h2q = sp.tile([P, DOUT], BF16, tag="h2q")
                    nc.scalar.activation(h2q[:], pm[:], AF.Copy)
                    ha = sp.tile([P, DOUT], BF16, tag="ha")
                    nc.vector.tensor_scalar_min(out=ha[:], in0=h2q[:],
                                                scalar1=0.0)
                    hb = sp.tile([P, DOUT], BF16, tag="hb")
                    nc.scalar.activation(hb[:], ha[:], AF.Exp)
                    h2p = sp.tile([P, DOUT], F32, tag="h2p")
                    nc.vector.scalar_tensor_tensor(
                        out=h2p[:], in0=hb[:], scalar=-1.0, in1=h2q[:],
                        op0=OP.add, op1=OP.max)
                    rmax = sp.tile([P, 1], F32, tag="rmax")
                    nc.vector.tensor_reduce(out=rmax[:], in_=h2p[:], axis=AX,
                                            op=OP.max)
                    z = sp.tile([P, DOUT], F32, tag="z")
                    nc.vector.tensor_scalar_sub(out=z[:], in0=h2p[:],
                                                scalar1=rmax[:])
                    ez = sp.tile([P, DOUT], F32, tag="ez")
                    ssum = sp.tile([P, 1], F32, tag="ssum")
                    nc.scalar.activation(ez[:], z[:], AF.Exp,
                                         accum_out=ssum[:])
                    lnz = sp.tile([P, 1], F32, tag="lnz")
                    nc.scalar.activation(lnz[:], ssum[:], AF.Ln)
                    outt = sp.tile([P, DOUT], F32, tag="outt")
                    nc.vector.tensor_scalar_sub(out=outt[:], in0=z[:],
                                                scalar1=lnz[:])
                    nc.sync.dma_start(out=OUT[nt * P:(nt + 1) * P, :],
                                      in_=outt[:])

    nc.finalize()
    return nc


def preprocess(X, edge_index, edge_attr, W_heads, a_heads, W_out, a_out):
    """Host-side layout prep. Returns (in_maps, meta)."""
    import ml_dtypes
    BF = ml_dtypes.bfloat16
    N = X.shape[0]
    E = edge_index.shape[1]
    NP = ((N + N_CORES * P - 1) // (N_CORES * P)) * (N_CORES * P)
    NSH = NP // N_CORES
    NT = NSH // P
    NT_ALL = NP // P
    NCH = (NP + CHROWS - 1) // CHROWS

    src = np.asarray(edge_index[0], dtype=np.int64)
    tgt = np.asarray(edge_index[1], dtype=np.int64)
    chunk_e = (tgt // CHROWS).astype(np.int64)

    deg = np.zeros((NCH, NP), np.int64)
    for k in range(NCH):
        deg[k] = np.bincount(src[chunk_e == k], minlength=NP)
    degt = deg.reshape(NCH, NT_ALL, P)

    cks, mks = [], []
    for k in range(NCH):
        best = None
        for cc in range(0, 40):
            ovv = np.maximum(degt[k] - cc, 0).sum(axis=1).max()
            mm = int(np.ceil(ovv / P))
            if cc == 0 and mm == 0:
                continue
            key = (cc + mm, mm, -cc)
            if best is None or key < best[0]:
                best = (key, cc, mm)
        cks.append(best[1])
        mks.append(best[2])
    while sum(mks) > 4:
        k = int(np.argmax(mks))
        cks[k] += 1
        ovv = np.maximum(degt[k] - cks[k], 0).sum(axis=1).max()
        mks[k] = int(np.ceil(ovv / P))
    GDEF, M = _gdef(cks, mks)
    G = len(GDEF)
    cbase = np.cumsum([0] + [cks[k] + mks[k] for k in range(NCH)])

    order = np.lexsort((chunk_e, src))
    s_s = src[order]
    t_s = tgt[order]
    k_s = chunk_e[order]
    ea_s = np.asarray(edge_attr, np.float32)[order]
    keyid = s_s * NCH + k_s
    cnt = np.bincount(keyid, minlength=NP * NCH)
    kstart = np.concatenate([[0], np.cumsum(cnt)])
    pos = np.arange(E) - kstart[keyid]

    core_e = (s_s // NSH).astype(np.int64)
    tl_e = ((s_s % NSH) // P).astype(np.int64)
    p_e = (s_s % P).astype(np.int64)

    ckarr = np.asarray(cks)[k_s]
    cbarr = cbase[k_s]

    idxlin = np.zeros((N_CORES, NT, G, P), np.int16)
    srclm = np.full((N_CORES, P, NT * M), -1.0, BF)
    srowm = np.full((N_CORES, NT, 1, M * P), -1.0, BF)

    ah = np.asarray(a_heads, np.float32)
    ao = np.asarray(a_out, np.float32)
    AE1m = ah[:, 2 * DH:2 * DH + EA]
    AE2m = ao[2 * DOUT:2 * DOUT + EA][None, :]
    A = np.concatenate([AE1m, AE2m], axis=0)
    v_pad = np.linalg.lstsq(A, np.full(H + 1, PAD_SCORE, np.float64),
                            rcond=None)[0].astype(np.float32)

    eatt = np.empty((N_CORES, EA, NT * G * P), np.float32)
    eatt[:] = v_pad[None, :, None]

    relt = (t_s - k_s * CHROWS).astype(np.int16)

    idm = pos < ckarr
    col_i = cbarr[idm] + pos[idm]
    idxlin[core_e[idm], tl_e[idm], col_i, p_e[idm]] = relt[idm]
    eatt[core_e[idm], :, (tl_e[idm] * G + col_i) * P + p_e[idm]] = ea_s[idm]

    ovm = ~idm
    if ovm.any():
        tidk = (s_s[ovm] // P) * NCH + k_s[ovm]
        ordm = np.argsort(tidk, kind="stable")
        tidk = tidk[ordm]
        sm_core = core_e[ovm][ordm]
        sm_tl = tl_e[ovm][ordm]
        sm_p = p_e[ovm][ordm]
        sm_k = k_s[ovm][ordm]
        sm_relt = relt[ovm][ordm]
        sm_ea = ea_s[ovm][ordm]
        tcnt = np.bincount(tidk, minlength=NT_ALL * NCH)
        tstart = np.concatenate([[0], np.cumsum(tcnt)])
        kk = np.arange(len(tidk)) - tstart[tidk]
        grp = kk // P
        lane = kk % P
        mkarr = np.asarray(mks)[sm_k]
        assert (grp < mkarr).all(), "masked group overflow"
        colm = cbase[sm_k] + np.asarray(cks)[sm_k] + grp
        idxlin[sm_core, sm_tl, colm, lane] = sm_relt
        eatt[sm_core, :, (sm_tl * G + colm) * P + lane] = sm_ea
        kmbase = np.zeros(NCH, np.int64)
        acc = 0
        for k in range(NCH):
            kmbase[k] = acc
            acc += mks[k]
        kmg = kmbase[sm_k] + grp
        srclm[sm_core, lane, sm_tl * M + kmg] = sm_p.astype(BF)
        srowm[sm_core, sm_tl, 0, kmg * P + lane] = sm_p.astype(BF)

    eatt = eatt.astype(BF)

    idx16 = np.zeros((N_CORES, P, NT * G * 8), np.int16)
    lin = idxlin.reshape(N_CORES, NT, G * P)
    tgtv = idx16.reshape(N_CORES, P, NT, G * 8)
    for k in range(NCH):
        gk = cks[k] + mks[k]
        if gk == 0:
            continue
        cb = int(cbase[k])
        A_ = lin[:, :, cb * P:(cb + gk) * P]
        W_ = A_.reshape(N_CORES, NT, gk * 8, 16).transpose(0, 3, 1, 2)
        for r in range(8):
            tgtv[:, r * 16:(r + 1) * 16, :, cb * 8:(cb + gk) * 8] = W_

    Xp = np.zeros((NP, DIN), np.float32)
    Xp[:N] = np.asarray(X, np.float32)
    XTT = np.ascontiguousarray(
        Xp.reshape(NT_ALL, P, DIN).transpose(0, 2, 1)).astype(BF)

    Wh = np.asarray(W_heads, np.float32)
    Wo = np.asarray(W_out, np.float32)
    WC = np.ascontiguousarray(Wh.transpose(1, 2, 0).reshape(DIN, DC))
    WAT = np.einsum("hkj,hj->kh", Wh, ah[:, DH:2 * DH])
    WAS = np.einsum("hkj,hj->kh", Wh, ah[:, :DH])
    AE1 = np.ascontiguousarray(AE1m.T)
    AE2 = np.ascontiguousarray(AE2m.T)
    base = np.concatenate(
        [Wo, (Wo @ ao[DOUT:2 * DOUT])[:, None], (Wo @ ao[:DOUT])[:, None]],
        axis=1).astype(np.float32)
    perm = (np.arange(DC) % 8) * DH + np.arange(DC) // 8
    W2X = np.ascontiguousarray(
        base[perm].reshape(4, P, 130).transpose(1, 0, 2).reshape(P, 4 * 130))
    IOTP = np.arange(P, dtype=np.float32)[:, None]
    IOTF = np.tile(np.arange(P, dtype=np.float32)[None, :], (P, 1))
    IDT = np.eye(P, dtype=np.float32)
    ON1 = np.ones((1, P), np.float32)

    in_maps = []
    for cidx in range(N_CORES):
        in_maps.append({
            "xtt": XTT,
            "xto": XTT[cidx * NT:(cidx + 1) * NT],
            "wc": WC.astype(BF), "wat": WAT.astype(BF),
            "was": WAS.astype(BF),
            "ae1": AE1.astype(BF), "ae2": AE2.astype(BF),
            "w2x": W2X.astype(BF),
            "iotp": IOTP.astype(BF), "iotf": IOTF.astype(BF),
            "idt": IDT.astype(BF), "on1": ON1.astype(BF),
            "idx16": idx16[cidx], "srclm": srclm[cidx],
            "srowm": srowm[cidx], "eatt": eatt[cidx],
        })
    meta = dict(N=N, NP=NP, cks=tuple(cks), mks=tuple(mks))
    return in_maps, meta


def make_runner(nc, n_cores=N_CORES):
    """Reusable jitted SPMD executor for a finalized Bass module."""
    import time
    import jax
    from jax.sharding import Mesh, PartitionSpec
    from jax.experimental.shard_map import shard_map
    from concourse import bass2jax
    from concourse.bass2jax import _bass_exec_p, partition_id_tensor

    bass2jax.install_neuronx_cc_hook()
    partition_name = (nc.partition_id_tensor.name
                      if nc.partition_id_tensor else None)
    in_names, out_names, out_avals, zero_outs = [], [], [], []
    for alloc in nc.m.functions[0].allocations:
        if not isinstance(alloc, mybir.MemoryLocationSet):
            continue
        name = alloc.memorylocations[0].name
        if alloc.kind == "ExternalInput":
            if name != partition_name:
                in_names.append(name)
        elif alloc.kind == "ExternalOutput":
            out_names.append(name)
            shape = tuple(alloc.tensor_shape)
            dtype = mybir.dt.np(alloc.dtype)
            out_avals.append(jax.core.ShapedArray(shape, dtype))
            zero_outs.append(np.zeros(shape, dtype))
    n_params = len(in_names)
    all_in_names = list(in_names) + list(out_names)
    if partition_name is not None:
        all_in_names.append(partition_name)

    def _body(*args):
        operands = list(args)
        if partition_name is not None:
            operands.append(partition_id_tensor())
        outs = _bass_exec_p.bind(
            *operands,
            out_avals=tuple(out_avals),
            in_names=tuple(all_in_names),
            out_names=tuple(out_names),
            lowering_input_output_aliases=(),
            sim_require_finite=True,
            sim_require_nnan=True,
            nc=nc,
        )
        return tuple(outs)

    devices = jax.devices()[:n_cores]
    mesh = Mesh(np.asarray(devices), ("core",))
    in_specs = (PartitionSpec("core"),) * (n_params + len(out_names))
    out_specs = (PartitionSpec("core"),) * len(out_names)
    sharded = jax.jit(
        shard_map(_body, mesh=mesh, in_specs=in_specs, out_specs=out_specs,
                  check_rep=False),
        keep_unused=True,
    )

    def run(in_maps, n_iters=0):
        per_core = [[np.asarray(mm[name]) for name in in_names]
                    for mm in in_maps]
        concat_in = [
            np.concatenate([per_core[cc][i] for cc in range(n_cores)], axis=0)
            for i in range(n_params)
        ]
        concat_zeros = [
            np.zeros((n_cores * z.shape[0], *z.shape[1:]), z.dtype)
            for z in zero_outs
        ]
        args = [jax.device_put(a) for a in concat_in]
        args += [jax.device_put(a) for a in concat_zeros]
        out = sharded(*args)
        jax.block_until_ready(out)
        dt = float("nan")
        if n_iters:
            t0 = time.perf_counter()
            for _ in range(n_iters):
                out = sharded(*args)
                jax.block_until_ready(out)
            dt = (time.perf_counter() - t0) / n_iters
        results = [
            {
                name: np.asarray(out[i]).reshape(
                    n_cores, *out_avals[i].shape)[cc]
                for i, name in enumerate(out_names)
            }
            for cc in range(n_cores)
        ]
        return results, dt

    return run


_RUNNER_CACHE = {}


def _get_runner(NP, cks, mks):
    key = (NP, tuple(cks), tuple(mks))
    if key not in _RUNNER_CACHE:
        nc = build_program(NP, cks, mks)
        _RUNNER_CACHE[key] = make_runner(nc, N_CORES)
    return _RUNNER_CACHE[key]


def kernel(X, edge_index, edge_attr, W_heads, a_heads, W_out, a_out):
    in_maps, meta = preprocess(X, edge_index, edge_attr, W_heads, a_heads,
                               W_out, a_out)
    run = _get_runner(meta["NP"], meta["cks"], meta["mks"])
    results, _ = run(in_maps, n_iters=0)
    out = np.concatenate([results[cc]["out"] for cc in range(N_CORES)],
                         axis=0)
    return out[:meta["N"]].astype(np.float32)



# revision 8
# speedup vs baseline: 20.2152x; 20.2152x over previous
"""Trainium2 Bass kernel v2 for the 2-layer multi-head GAT.

Design (8 NeuronCores, SPMD):
  - Gather tables: G1[v] = [h(v) (d,h)-interleaved (512) | s_tgt (8) | pad]
    as 640 bf16 = 1280B rows; G2[v] = [h2 fp8 (128) | s_tgt2 bf16 | s_src2
    bf16 | pad] as 256 fp8-bytes = 256B rows (fp8 payload halves the
    AllGather).
  - Edge gathers via dma_gather (Ant SWDGE custom op): int16 indices force a
    split of the node space into <=32768-row chunks; one dma_gather per
    (node-tile, chunk) pulls that chunk's edge slots (group-padded to 128).
  - Per (tile, chunk): c_k "identity layers" (layer j, lane p = node p's j-th
    chunk-k edge; segment-sum = PSUM accumulate with identity stationary) +
    m_k masked overflow groups (maskT matmul).
  - Pad slots gather chunk row 0 and carry a host-crafted edge-attr column
    v_pad whose (ae1|ae2)-projections are -1e4, so exp(lrelu(score))
    underflows to 0 in bf16 and pads contribute exactly nothing.
  - Scores batched per tile; exp on ScalarE; attention normalized BEFORE the
    message multiply (exn = ex * rcp[src]); message mult at DVE 2x bf16 via
    the (d,h) interleave (broadcast over d keeps the last dim packed).
  - Layer-2 h2 = x1 @ W2X via 4 PE transposes (copies on ScalarE); AllGather
    of the fp8 G2 shard; same edge machinery with 1 head; fused log_softmax.
"""

import numpy as np

import concourse.bass as bass
import concourse.bacc as bacc
import concourse.mybir as mybir
import concourse.tile as tile

F32 = mybir.dt.float32
BF16 = mybir.dt.bfloat16
FP8 = mybir.dt.float8e4
I16 = mybir.dt.int16

N_CORES = 8
P = 128
H = 8
DH = 64
DIN = 128
DC = 512
DOUT = 128
EA = 16
LRELU = 0.01
G1W = 640         # row elems (bf16): [h 512 | s_tgt 8 | pad]
G1B = 520         # written row body
G2W = 256         # row bytes (fp8 dtype): [h2 fp8 128 | stgt2 bf16 | ssrc2]
CHROWS = 32768
EPS0 = 1e-30
PAD_SCORE = -1.0e4


def _gdef(cks, mks):
    """Group definitions: list of (chunk, is_identity, masked_idx)."""
    gdef = []
    km = 0
    for k, (ck, mk) in enumerate(zip(cks, mks)):
        for _ in range(ck):
            gdef.append((k, True, None))
        for _ in range(mk):
            gdef.append((k, False, km))
            km += 1
    return gdef, km


def build_program(NP, cks, mks, repeat=1, dbg=0, phases=4, e1sub=0, bench=0):
    cks, mks = tuple(cks), tuple(mks)
    GDEF, M = _gdef(cks, mks)
    G = len(GDEF)
    NCH = len(cks)
    cbase = np.cumsum([0] + [cks[k] + mks[k] for k in range(NCH)])
    chbase = [k * CHROWS for k in range(NCH)]
    chsize = [min(CHROWS, NP - k * CHROWS) for k in range(NCH)]
    id_runs = [(int(cbase[k]), cks[k]) for k in range(NCH) if cks[k] > 0]
    mk_runs = []
    km = 0
    for k in range(NCH):
        if mks[k] > 0:
            mk_runs.append((int(cbase[k]) + cks[k], km, mks[k]))
            km += mks[k]

    NT_ALL = NP // P
    NSH = NP // N_CORES
    NT = NSH // P

    nc = bacc.Bacc("TRN2", target_bir_lowering=False, debug=False,
                   num_devices=N_CORES)

    XTT = nc.dram_tensor("xtt", [NT_ALL, DIN, P], BF16, kind="ExternalInput")
    XTO = nc.dram_tensor("xto", [NT, DIN, P], BF16, kind="ExternalInput")
    WC = nc.dram_tensor("wc", [DIN, DC], BF16, kind="ExternalInput")
    WAT = nc.dram_tensor("wat", [DIN, H], BF16, kind="ExternalInput")
    WAS = nc.dram_tensor("was", [DIN, H], BF16, kind="ExternalInput")
    AE1 = nc.dram_tensor("ae1", [EA, H], BF16, kind="ExternalInput")
    AE2 = nc.dram_tensor("ae2", [EA, 1], BF16, kind="ExternalInput")
    W2X = nc.dram_tensor("w2x", [P, 4 * 130], BF16, kind="ExternalInput")
    IOTP = nc.dram_tensor("iotp", [P, 1], BF16, kind="ExternalInput")
    IOTF = nc.dram_tensor("iotf", [P, P], BF16, kind="ExternalInput")
    IDT = nc.dram_tensor("idt", [P, P], BF16, kind="ExternalInput")
    ON1 = nc.dram_tensor("on1", [1, P], BF16, kind="ExternalInput")
    IDX16 = nc.dram_tensor("idx16", [P, NT * G * 8], I16,
                           kind="ExternalInput")
    SRCLM = nc.dram_tensor("srclm", [P, NT * M], BF16, kind="ExternalInput")
    SROWM = nc.dram_tensor("srowm", [NT, 1, M * P], BF16,
                           kind="ExternalInput")
    EATT = nc.dram_tensor("eatt", [EA, NT * G * P], BF16,
                          kind="ExternalInput")

    G1 = nc.dram_tensor("g1", [NP, G1W], BF16)
    G2S = nc.dram_tensor("g2s", [NSH, G2W], FP8)
    G2F = nc.dram_tensor("g2f", [NP, G2W], FP8, addr_space="Shared")

    OUT = nc.dram_tensor("out", [NSH, DOUT], F32, kind="ExternalOutput")
    if dbg:
        G1D = nc.dram_tensor("g1d", [dbg, G1W], BF16, kind="ExternalOutput")
        G2SD = nc.dram_tensor("g2sd", [NSH, G2W], FP8, kind="ExternalOutput")

    AX = mybir.AxisListType.X
    OP = mybir.AluOpType
    AF = mybir.ActivationFunctionType

    with tile.TileContext(nc) as tc, \
         tc.tile_pool(name="const", bufs=1) as cp:
        wc_sb = cp.tile([DIN, DC], BF16, tag="wc")
        nc.sync.dma_start(out=wc_sb[:], in_=WC[:])
        wat_sb = cp.tile([DIN, H], BF16, tag="wat")
        nc.sync.dma_start(out=wat_sb[:], in_=WAT[:])
        was_sb = cp.tile([DIN, H], BF16, tag="was")
        nc.sync.dma_start(out=was_sb[:], in_=WAS[:])
        ae1_sb = cp.tile([EA, H], BF16, tag="ae1")
        nc.sync.dma_start(out=ae1_sb[:], in_=AE1[:])
        ae2_sb = cp.tile([EA, 1], BF16, tag="ae2")
        nc.sync.dma_start(out=ae2_sb[:], in_=AE2[:])
        w2x_sb = cp.tile([P, 4 * 130], BF16, tag="w2x")
        nc.sync.dma_start(out=w2x_sb[:], in_=W2X[:])
        iotp_sb = cp.tile([P, 1], BF16, tag="iotp")
        nc.sync.dma_start(out=iotp_sb[:], in_=IOTP[:])
        iotf_sb = cp.tile([P, P], BF16, tag="iotf")
        nc.sync.dma_start(out=iotf_sb[:], in_=IOTF[:])
        idt_sb = cp.tile([P, P], BF16, tag="idt")
        nc.sync.dma_start(out=idt_sb[:], in_=IDT[:])
        on1_sb = cp.tile([1, P], BF16, tag="on1")
        nc.sync.dma_start(out=on1_sb[:], in_=ON1[:])
        idx_sb = cp.tile([P, NT * G * 8], I16, tag="idx16")
        nc.sync.dma_start(out=idx_sb[:], in_=IDX16[:])
        srclm_sb = cp.tile([P, NT * M], BF16, tag="srclm")
        nc.sync.dma_start(out=srclm_sb[:], in_=SRCLM[:])
        s2all_sb = cp.tile([P, NT], BF16, tag="s2all")
        neg1_sb = cp.tile([P, 1], F32, tag="neg1")
        nc.vector.memset(neg1_sb[:], -1.0)

        for _rep in range(repeat):
            if _rep:
                tc.strict_bb_all_engine_barrier()

            # ---------------- Phase D1: build G1 table (replicated) --------
            with tc.tile_pool(name="d1", bufs=4) as dp, \
                 tc.tile_pool(name="d1ps", bufs=4, space="PSUM") as dps:
                B4 = 4
                for ib in range(0, NT_ALL, B4):
                    nb = min(B4, NT_ALL - ib)
                    xt = dp.tile([P, B4 * DIN], BF16, tag="xt")
                    nc.sync.dma_start(
                        out=xt[:].rearrange("p (b k) -> p b k", b=B4)[:, 0:nb],
                        in_=XTT[ib:ib + nb].rearrange("b k p -> k b p"))
                    g1t = dp.tile([P, B4 * G1B], BF16, tag="g1t")
                    for k in range(nb):
                        i = ib + k
                        ph = dps.tile([P, DC], F32, tag="ph")
                        nc.tensor.matmul(ph[:], xt[:, k * DIN:(k + 1) * DIN],
                                         wc_sb[:], start=True, stop=True)
                        ps = dps.tile([P, H], F32, tag="ps")
                        nc.tensor.matmul(ps[:], xt[:, k * DIN:(k + 1) * DIN],
                                         wat_sb[:], start=True, stop=True)
                        gs = g1t[:, k * G1B:(k + 1) * G1B]
                        eng = (nc.vector, nc.scalar)[i % 2]
                        if eng is nc.scalar:
                            nc.scalar.activation(gs[:, 0:DC], ph[:], AF.Copy)
                            nc.scalar.activation(gs[:, DC:G1B], ps[:],
                                                 AF.Copy)
                        else:
                            eng.tensor_copy(out=gs[:, 0:DC], in_=ph[:])
                            eng.tensor_copy(out=gs[:, DC:G1B], in_=ps[:])
                    nc.sync.dma_start(
                        out=G1[ib * P:(ib + nb) * P, 0:G1B]
                            .rearrange("(b p) w -> p b w", p=P),
                        in_=g1t[:].rearrange("p (b w) -> p b w", b=B4)
                            [:, 0:nb])

            tc.strict_bb_all_engine_barrier()

            if bench:
                BPROBES = [(640, 1, 1024)]
                with tc.tile_pool(name="bench", bufs=2) as bp, \
                     tc.tile_pool(name="bacc", bufs=1) as bap:
                    acc = bap.tile([P, 64], F32, tag="acc")
                    nc.vector.memset(acc[:], 0.0)
                    kk = 0
                    for _r in range(3):
                        for (es, tp, ni) in BPROBES:
                            dst = bp.tile([P, 56 * 640], BF16, tag="bd")
                            if tp:
                                no = es // 128
                                nc.gpsimd.dma_gather(
                                    dst[:, 0:ni * no]
                                        .rearrange("p (o n) -> p o n", o=no),
                                    G1[0:CHROWS, 0:es],
                                    idx_sb[:, 0:ni // 16], ni, ni, es,
                                    elem_step=G1W, transpose=True)
                                used = ni * no
                            else:
                                ncol = ni // 128
                                nc.gpsimd.dma_gather(
                                    dst[:, 0:ncol * es]
                                        .rearrange("p (g w) -> p g w", w=es),
                                    G1[0:CHROWS, 0:es],
                                    idx_sb[:, 0:ni // 16], ni, ni, es,
                                    elem_step=G1W)
                                used = ncol * es
                            nc.vector.tensor_reduce(
                                out=acc[:, kk:kk + 1], in_=dst[:, 0:used],
                                axis=AX, op=OP.max)
                            kk += 1
                    nc.sync.dma_start(out=OUT[0:P, 0:64], in_=acc[:])
                tc.strict_bb_all_engine_barrier()

            if dbg:
                with tc.tile_pool(name="dbg1", bufs=2) as dbp:
                    for i in range(dbg // P):
                        t = dbp.tile([P, G1W], BF16, tag="t")
                        nc.sync.dma_start(out=t[:],
                                          in_=G1[i * P:(i + 1) * P, :])
                        nc.sync.dma_start(out=G1D[i * P:(i + 1) * P, :],
                                          in_=t[:])
                tc.strict_bb_all_engine_barrier()

            # ---------------- Phase E1: layer-1 edge pass ------------------
            if phases < 2:
                continue
            with tc.tile_pool(name="e1g", bufs=3) as gp, \
                 tc.tile_pool(name="e1w", bufs=2) as ep, \
                 tc.tile_pool(name="e1s", bufs=2) as sp, \
                 tc.tile_pool(name="e1r", bufs=4) as rp, \
                 tc.tile_pool(name="e1x", bufs=2) as xp, \
                 tc.tile_pool(name="psA", bufs=2, space="PSUM") as psA, \
                 tc.tile_pool(name="psB", bufs=1, space="PSUM") as psB, \
                 tc.tile_pool(name="psM", bufs=2, space="PSUM") as psM, \
                 tc.tile_pool(name="psT", bufs=2, space="PSUM") as psT, \
                 tc.tile_pool(name="psG", bufs=1, space="PSUM") as psG:
                for nt in range(NT):
                    gdst = gp.tile([P, G * G1W], BF16, tag="g")
                    for k in range(NCH):
                        gk = cks[k] + mks[k]
                        cb = int(cbase[k])
                        off = 0
                        while off < gk:
                            gn = min(8, gk - off)
                            c0 = cb + off
                            nc.gpsimd.dma_gather(
                                gdst[:, c0 * G1W:(c0 + gn) * G1W]
                                    .rearrange("p (g w) -> p g w", w=G1W),
                                G1[chbase[k]:chbase[k] + chsize[k], :],
                                idx_sb[:, (nt * G + c0) * 8:
                                       (nt * G + c0 + gn) * 8],
                                gn * P,
                                gn * P,
                                G1W,
                            )
                            off += gn
                    eat = ep.tile([EA, G * P], BF16, tag="eat")
                    nc.sync.dma_start(
                        out=eat[:], in_=EATT[:, nt * G * P:(nt + 1) * G * P])
                    xto = ep.tile([P, DIN], BF16, tag="xto")
                    nc.sync.dma_start(out=xto[:], in_=XTO[nt])
                    srow = ep.tile([1, M * P], BF16, tag="srow")
                    nc.sync.dma_start(out=srow[:], in_=SROWM[nt])
                    if e1sub == 1:
                        continue

                    NV = 8 + G * 8 + 8 + M * 8
                    psa = psA.tile([P, NV], F32, tag="a")
                    s1p = psa[:, 0:8]
                    pv = psa[:, 8:8 + G * 8]
                    pd = psa[:, 8 + G * 8:16 + G * 8]
                    prg = psa[:, 16 + G * 8:NV]
                    pbc = psB.tile([P, M * P], F32, tag="b")

                    nc.tensor.matmul(s1p[:], xto[:], was_sb[:],
                                     start=True, stop=True)
                    s1sb = sp.tile([P, H], BF16, tag="s1sb")
                    nc.vector.tensor_copy(out=s1sb[:], in_=s1p[:])

                    nc.tensor.matmul(
                        pbc[:], on1_sb[:], srow[0:1, :],
                        start=True, stop=True)
                    maskv = sp.tile([P, M * P], BF16, tag="maskv")
                    nc.vector.tensor_tensor(
                        out=maskv[:], in0=pbc[:],
                        in1=iotp_sb[:].to_broadcast([P, M * P]),
                        op=OP.is_equal)
                    maskT = sp.tile([P, M * P], BF16, tag="maskT")
                    for km in range(M):
                        nc.vector.tensor_tensor(
                            out=maskT[:, km * P:(km + 1) * P],
                            in0=srclm_sb[:, nt * M + km:nt * M + km + 1]
                                .to_broadcast([P, P]),
                            in1=iotf_sb[:], op=OP.is_equal)

                    for j, (k, is_id, km) in enumerate(GDEF):
                        nc.tensor.matmul(
                            pv[:, j * 8:(j + 1) * 8],
                            eat[:, j * P:(j + 1) * P], ae1_sb[:],
                            start=True, stop=bool(is_id))
                        if not is_id:
                            nc.tensor.matmul(
                                pv[:, j * 8:(j + 1) * 8],
                                maskv[:, km * P:(km + 1) * P], s1sb[:],
                                start=False, stop=True)
                    sc = sp.tile([P, G * 8], BF16, tag="sc")
                    nc.vector.tensor_tensor(
                        out=sc[:].rearrange("p (g h) -> p g h", h=8),
                        in0=pv.rearrange("p (g h) -> p g h", h=8),
                        in1=gdst[:].rearrange("p (g w) -> p g w", w=G1W)
                            [:, :, DC:DC + 8],
                        op=OP.add)
                    for (c0, cn) in id_runs:
                        nc.vector.tensor_tensor(
                            out=sc[:, c0 * 8:(c0 + cn) * 8]
                                .rearrange("p (g h) -> p g h", h=8),
                            in0=sc[:, c0 * 8:(c0 + cn) * 8]
                                .rearrange("p (g h) -> p g h", h=8),
                            in1=s1sb[:].unsqueeze(1).to_broadcast([P, cn, 8]),
                            op=OP.add)
                    lr = sp.tile([P, G * 8], BF16, tag="lr")
                    nc.vector.scalar_tensor_tensor(
                        out=lr[:], in0=sc[:], scalar=LRELU, in1=sc[:],
                        op0=OP.mult, op1=OP.max)
                    ex = sp.tile([P, G * 8], BF16, tag="ex")
                    nc.scalar.activation(ex[:], lr[:], AF.Exp)
                    if e1sub == 2:
                        continue

                    for j, (k, is_id, km) in enumerate(GDEF):
                        lhs = idt_sb[:] if is_id else \
                            maskT[:, km * P:(km + 1) * P]
                        nc.tensor.matmul(pd[:], lhs, ex[:, j * 8:(j + 1) * 8],
                                         start=(j == 0), stop=(j == G - 1))
                    den = sp.tile([P, H], BF16, tag="den")
                    nc.vector.tensor_scalar_add(out=den[:], in0=pd[:],
                                                scalar1=EPS0)
                    rcp = sp.tile([P, H], BF16, tag="rcp")
                    with nc.allow_low_precision(reason="attn rcp bf16 ok"):
                        nc.vector.reciprocal(out=rcp[:], in_=den[:])
                    exn = sp.tile([P, G * 8], BF16, tag="exn")
                    for (c0, cn) in id_runs:
                        nc.vector.tensor_tensor(
                            out=exn[:, c0 * 8:(c0 + cn) * 8]
                                .rearrange("p (g h) -> p g h", h=8),
                            in0=ex[:, c0 * 8:(c0 + cn) * 8]
                                .rearrange("p (g h) -> p g h", h=8),
                            in1=rcp[:].unsqueeze(1).to_broadcast([P, cn, 8]),
                            op=OP.mult)
                    for km in range(M):
                        nc.tensor.matmul(
                            prg[:, km * 8:(km + 1) * 8],
                            maskv[:, km * P:(km + 1) * P], rcp[:],
                            start=(km == 0), stop=(km == M - 1))
                    for (g0, km0, mn) in mk_runs:
                        nc.vector.tensor_tensor(
                            out=exn[:, g0 * 8:(g0 + mn) * 8],
                            in0=ex[:, g0 * 8:(g0 + mn) * 8],
                            in1=prg[:, km0 * 8:(km0 + mn) * 8], op=OP.mult)

                    pm = psM.tile([P, DC], F32, tag="pm")
                    for j, (k, is_id, km) in enumerate(GDEF):
                        rhs = rp.tile([P, DC], BF16, tag="rhs")
                        nc.vector.tensor_tensor(
                            out=rhs[:].rearrange("p (d h) -> p d h", h=8),
                            in0=gdst[:, j * G1W:j * G1W + DC]
                                .rearrange("p (d h) -> p d h", h=8),
                            in1=exn[:, j * 8:(j + 1) * 8].unsqueeze(1)
                                .to_broadcast([P, DH, 8]),
                            op=OP.mult)
                        lhs = idt_sb[:] if is_id else \
                            maskT[:, km * P:(km + 1) * P]
                        nc.tensor.matmul(pm[:], lhs, rhs[:],
                                         start=(j == 0), stop=(j == G - 1))

                    x0 = xp.tile([P, DC], BF16, tag="x0")
                    nc.scalar.activation(x0[:], pm[:], AF.Copy)
                    if e1sub == 3:
                        continue
                    xa = xp.tile([P, DC], BF16, tag="xa")
                    nc.vector.tensor_scalar_min(out=xa[:], in0=x0[:],
                                                scalar1=0.0)
                    xb = xp.tile([P, DC], BF16, tag="xb")
                    nc.scalar.activation(xb[:], xa[:], AF.Exp)
                    xd = xp.tile([P, DC], BF16, tag="xd")
                    nc.scalar.activation(xd[:], xb[:], AF.Exp,
                                         bias=neg1_sb[:])
                    x1 = xp.tile([P, DC], BF16, tag="x1")
                    nc.vector.scalar_tensor_tensor(
                        out=x1[:], in0=xd[:], scalar=-1.0, in1=x0[:],
                        op0=OP.add, op1=OP.max)
                    pg2 = psG.tile([P, 130], F32, tag="pg2")
                    for c4 in range(4):
                        pxT = psT.tile([P, P], BF16, tag="pxT")
                        nc.tensor.transpose(pxT[:],
                                            x1[:, c4 * P:(c4 + 1) * P],
                                            idt_sb[:])
                        xTs = rp.tile([P, P], BF16, tag="xTs")
                        nc.scalar.activation(xTs[:], pxT[:], AF.Copy)
                        nc.tensor.matmul(
                            pg2[:], xTs[:],
                            w2x_sb[:, c4 * 130:(c4 + 1) * 130],
                            start=(c4 == 0), stop=(c4 == 3))
                    g2t = sp.tile([P, G2W], FP8, tag="g2t")
                    nc.vector.tensor_copy(out=g2t[:, 0:DOUT],
                                          in_=pg2[:, 0:DOUT])
                    nc.vector.tensor_copy(
                        out=g2t[:].bitcast(BF16)[:, 64:66],
                        in_=pg2[:, DOUT:DOUT + 2])
                    nc.vector.tensor_copy(out=s2all_sb[:, nt:nt + 1],
                                          in_=pg2[:, 129:130])
                    nc.sync.dma_start(out=G2S[nt * P:(nt + 1) * P, :],
                                      in_=g2t[:])

            tc.strict_bb_all_engine_barrier()

            # ---------------- AllGather G2 shard -> full table -------------
            if phases < 3:
                continue
            with tc.tile_critical():
                with nc.semaphore() as cc_sem:
                    nc.gpsimd.collective_compute(
                        "AllGather", OP.bypass,
                        replica_groups=[list(range(N_CORES))],
                        ins=[G2S[:]], outs=[G2F[0:NP, :]],
                    ).then_inc(cc_sem, 1)
                    nc.gpsimd.wait_ge(cc_sem, 1)

            tc.strict_bb_all_engine_barrier()

            if dbg:
                with tc.tile_pool(name="dbg2", bufs=2) as dbp:
                    for i in range(NSH // P):
                        t = dbp.tile([P, G2W], FP8, tag="t")
                        nc.sync.dma_start(out=t[:],
                                          in_=G2S[i * P:(i + 1) * P, :])
                        nc.sync.dma_start(out=G2SD[i * P:(i + 1) * P, :],
                                          in_=t[:])
                tc.strict_bb_all_engine_barrier()

            # ---------------- Phase E2: layer-2 edge pass + log_softmax ----
            if phases < 4:
                continue
            with tc.tile_pool(name="e2g", bufs=3) as gp, \
                 tc.tile_pool(name="e2w", bufs=2) as ep, \
                 tc.tile_pool(name="e2s", bufs=2) as sp, \
                 tc.tile_pool(name="e2r", bufs=4) as rp, \
                 tc.tile_pool(name="psA2", bufs=2, space="PSUM") as psA, \
                 tc.tile_pool(name="psB2", bufs=1, space="PSUM") as psB, \
                 tc.tile_pool(name="psM2", bufs=2, space="PSUM") as psM:
                for nt in range(NT):
                    gdst = gp.tile([P, G * G2W], FP8, tag="g2")
                    for k in range(NCH):
                        gk = cks[k] + mks[k]
                        cb = int(cbase[k])
                        off = 0
                        while off < gk:
                            gn = min(8, gk - off)
                            c0 = cb + off
                            nc.gpsimd.dma_gather(
                                gdst[:, c0 * G2W:(c0 + gn) * G2W]
                                    .rearrange("p (g w) -> p g w", w=G2W),
                                G2F[chbase[k]:chbase[k] + chsize[k], :],
                                idx_sb[:, (nt * G + c0) * 8:
                                       (nt * G + c0 + gn) * 8],
                                gn * P,
                                gn * P,
                                G2W,
                            )
                            off += gn
                    eat = ep.tile([EA, G * P], BF16, tag="eat")
                    nc.sync.dma_start(
                        out=eat[:], in_=EATT[:, nt * G * P:(nt + 1) * G * P])
                    srow = ep.tile([1, M * P], BF16, tag="srow")
                    nc.sync.dma_start(out=srow[:], in_=SROWM[nt])

                    NV = G + 1 + M
                    psa = psA.tile([P, NV], F32, tag="a")
                    pv = psa[:, 0:G]
                    pd = psa[:, G:G + 1]
                    prg = psa[:, G + 1:NV]
                    pbc = psB.tile([P, M * P], F32, tag="b")

                    nc.tensor.matmul(
                        pbc[:], on1_sb[:], srow[0:1, :],
                        start=True, stop=True)
                    maskv = sp.tile([P, M * P], BF16, tag="maskv")
                    nc.vector.tensor_tensor(
                        out=maskv[:], in0=pbc[:],
                        in1=iotp_sb[:].to_broadcast([P, M * P]),
                        op=OP.is_equal)
                    maskT = sp.tile([P, M * P], BF16, tag="maskT")
                    for km in range(M):
                        nc.vector.tensor_tensor(
                            out=maskT[:, km * P:(km + 1) * P],
                            in0=srclm_sb[:, nt * M + km:nt * M + km + 1]
                                .to_broadcast([P, P]),
                            in1=iotf_sb[:], op=OP.is_equal)

                    for j, (k, is_id, km) in enumerate(GDEF):
                        nc.tensor.matmul(
                            pv[:, j:j + 1], eat[:, j * P:(j + 1) * P],
                            ae2_sb[:], start=True, stop=bool(is_id))
                        if not is_id:
                            nc.tensor.matmul(
                                pv[:, j:j + 1],
                                maskv[:, km * P:(km + 1) * P],
                                s2all_sb[:, nt:nt + 1],
                                start=False, stop=True)
                    sc = sp.tile([P, G], BF16, tag="sc")
                    nc.vector.tensor_tensor(
                        out=sc[:].unsqueeze(2), in0=pv.unsqueeze(2),
                        in1=gdst[:].bitcast(BF16)
                            .rearrange("p (g w) -> p g w", w=G2W // 2)
                            [:, :, 64:65],
                        op=OP.add)
                    for (c0, cn) in id_runs:
                        nc.vector.tensor_tensor(
                            out=sc[:, c0:c0 + cn], in0=sc[:, c0:c0 + cn],
                            in1=s2all_sb[:, nt:nt + 1].to_broadcast([P, cn]),
                            op=OP.add)
                    lr = sp.tile([P, G], BF16, tag="lr")
                    nc.vector.scalar_tensor_tensor(
                        out=lr[:], in0=sc[:], scalar=LRELU, in1=sc[:],
                        op0=OP.mult, op1=OP.max)
                    ex = sp.tile([P, G], BF16, tag="ex")
                    nc.scalar.activation(ex[:], lr[:], AF.Exp)
                    for j, (k, is_id, km) in enumerate(GDEF):
                        lhs = idt_sb[:] if is_id else \
                            maskT[:, km * P:(km + 1) * P]
                        nc.tensor.matmul(pd[:], lhs, ex[:, j:j + 1],
                                         start=(j == 0), stop=(j == G - 1))
                    den = sp.tile([P, 1], BF16, tag="den")
                    nc.vector.tensor_scalar_add(out=den[:], in0=pd[:],
                                                scalar1=EPS0)
                    rcp = sp.tile([P, 1], F32, tag="rcp")
                    with nc.allow_low_precision(reason="attn rcp ok"):
                        nc.vector.reciprocal(out=rcp[:], in_=den[:])
                    rcpb = sp.tile([P, 1], BF16, tag="rcpb")
                    nc.vector.tensor_copy(out=rcpb[:], in_=rcp[:])
                    exn = sp.tile([P, G], F32, tag="exn")
                    for (c0, cn) in id_runs:
                        nc.vector.tensor_tensor(
                            out=exn[:, c0:c0 + cn], in0=ex[:, c0:c0 + cn],
                            in1=rcp[:].to_broadcast([P, cn]), op=OP.mult)
                    for km in range(M):
                        nc.tensor.matmul(
                            prg[:, km:km + 1],
                            maskv[:, km * P:(km + 1) * P], rcpb[:],
                            start=(km == 0), stop=(km == M - 1))
                    for (g0, km0, mn) in mk_runs:
                        nc.vector.tensor_tensor(
                            out=exn[:, g0:g0 + mn], in0=ex[:, g0:g0 + mn],
                            in1=prg[:, km0:km0 + mn], op=OP.mult)

                    pm = psM.tile([P, DOUT], F32, tag="pm")
                    for j, (k, is_id, km) in enumerate(GDEF):
                        rhs = rp.tile([P, DOUT], BF16, tag="rhs")
                        nc.vector.tensor_scalar_mul(
                            out=rhs[:], in0=gdst[:, j * G2W:j * G2W + DOUT],
                            scalar1=exn[:, j:j + 1])
                        lhs = idt_sb[:] if is_id else \
                            maskT[:, km * P:(km + 1) * P]
                        nc.tensor.matmul(pm[:], lhs, rhs[:],
                                         start=(j == 0), stop=(j == G - 1))

                    h2q = sp.tile([P, DOUT], BF16, tag="h2q")
                    nc.scalar.activation(h2q[:], pm[:], AF.Copy)
                    ha = sp.tile([P, DOUT], BF16, tag="ha")
                    nc.vector.tensor_scalar_min(out=ha[:], in0=h2q[:],
                                                scalar1=0.0)
                    hb = sp.tile([P, DOUT], BF16, tag="hb")
                    nc.scalar.activation(hb[:], ha[:], AF.Exp)
                    h2p = sp.tile([P, DOUT], F32, tag="h2p")
                    nc.vector.scalar_tensor_tensor(
                        out=h2p[:], in0=hb[:], scalar=-1.0, in1=h2q[:],
                        op0=OP.add, op1=OP.max)
                    rmax = sp.tile([P, 1], F32, tag="rmax")
                    nc.vector.tensor_reduce(out=rmax[:], in_=h2p[:], axis=AX,
                                            op=OP.max)
                    z = sp.tile([P, DOUT], F32, tag="z")
                    nc.vector.tensor_scalar_sub(out=z[:], in0=h2p[:],
                                                scalar1=rmax[:])
                    ez = sp.tile([P, DOUT], F32, tag="ez")
                    ssum = sp.tile([P, 1], F32, tag="ssum")
                    nc.scalar.activation(ez[:], z[:], AF.Exp,
                                         accum_out=ssum[:])
                    lnz = sp.tile([P, 1], F32, tag="lnz")
                    nc.scalar.activation(lnz[:], ssum[:], AF.Ln)
                    outt = sp.tile([P, DOUT], F32, tag="outt")
                    nc.vector.tensor_scalar_sub(out=outt[:], in0=z[:],
                                                scalar1=lnz[:])
                    nc.sync.dma_start(out=OUT[nt * P:(nt + 1) * P, :],
                                      in_=outt[:])

    nc.finalize()
    return nc


def preprocess(X, edge_index, edge_attr, W_heads, a_heads, W_out, a_out):
    """Host-side layout prep. Returns (in_maps, meta)."""
    import ml_dtypes
    BF = ml_dtypes.bfloat16
    N = X.shape[0]
    E = edge_index.shape[1]
    NP = ((N + N_CORES * P - 1) // (N_CORES * P)) * (N_CORES * P)
    NSH = NP // N_CORES
    NT = NSH // P
    NT_ALL = NP // P
    NCH = (NP + CHROWS - 1) // CHROWS

    src = np.asarray(edge_index[0], dtype=np.int64)
    tgt = np.asarray(edge_index[1], dtype=np.int64)
    chunk_e = (tgt // CHROWS).astype(np.int64)

    deg = np.zeros((NCH, NP), np.int64)
    for k in range(NCH):
        deg[k] = np.bincount(src[chunk_e == k], minlength=NP)
    degt = deg.reshape(NCH, NT_ALL, P)

    cks, mks = [], []
    for k in range(NCH):
        best = None
        for cc in range(0, 40):
            ovv = np.maximum(degt[k] - cc, 0).sum(axis=1).max()
            mm = int(np.ceil(ovv / P))
            if cc == 0 and mm == 0:
                continue
            key = (cc + mm, mm, -cc)
            if best is None or key < best[0]:
                best = (key, cc, mm)
        cks.append(best[1])
        mks.append(best[2])
    while sum(mks) > 4:
        k = int(np.argmax(mks))
        cks[k] += 1
        ovv = np.maximum(degt[k] - cks[k], 0).sum(axis=1).max()
        mks[k] = int(np.ceil(ovv / P))
    GDEF, M = _gdef(cks, mks)
    G = len(GDEF)
    cbase = np.cumsum([0] + [cks[k] + mks[k] for k in range(NCH)])

    order = np.lexsort((chunk_e, src))
    s_s = src[order]
    t_s = tgt[order]
    k_s = chunk_e[order]
    ea_s = np.asarray(edge_attr, np.float32)[order]
    keyid = s_s * NCH + k_s
    cnt = np.bincount(keyid, minlength=NP * NCH)
    kstart = np.concatenate([[0], np.cumsum(cnt)])
    pos = np.arange(E) - kstart[keyid]

    core_e = (s_s // NSH).astype(np.int64)
    tl_e = ((s_s % NSH) // P).astype(np.int64)
    p_e = (s_s % P).astype(np.int64)

    ckarr = np.asarray(cks)[k_s]
    cbarr = cbase[k_s]

    idxlin = np.zeros((N_CORES, NT, G, P), np.int16)
    srclm = np.full((N_CORES, P, NT * M), -1.0, BF)
    srowm = np.full((N_CORES, NT, 1, M * P), -1.0, BF)

    ah = np.asarray(a_heads, np.float32)
    ao = np.asarray(a_out, np.float32)
    AE1m = ah[:, 2 * DH:2 * DH + EA]
    AE2m = ao[2 * DOUT:2 * DOUT + EA][None, :]
    A = np.concatenate([AE1m, AE2m], axis=0)
    v_pad = np.linalg.lstsq(A, np.full(H + 1, PAD_SCORE, np.float64),
                            rcond=None)[0].astype(np.float32)

    eatt = np.empty((N_CORES, EA, NT * G * P), np.float32)
    eatt[:] = v_pad[None, :, None]

    relt = (t_s - k_s * CHROWS).astype(np.int16)

    idm = pos < ckarr
    col_i = cbarr[idm] + pos[idm]
    idxlin[core_e[idm], tl_e[idm], col_i, p_e[idm]] = relt[idm]
    eatt[core_e[idm], :, (tl_e[idm] * G + col_i) * P + p_e[idm]] = ea_s[idm]

    ovm = ~idm
    if ovm.any():
        tidk = (s_s[ovm] // P) * NCH + k_s[ovm]
        ordm = np.argsort(tidk, kind="stable")
        tidk = tidk[ordm]
        sm_core = core_e[ovm][ordm]
        sm_tl = tl_e[ovm][ordm]
        sm_p = p_e[ovm][ordm]
        sm_k = k_s[ovm][ordm]
        sm_relt = relt[ovm][ordm]
        sm_ea = ea_s[ovm][ordm]
        tcnt = np.bincount(tidk, minlength=NT_ALL * NCH)
        tstart = np.concatenate([[0], np.cumsum(tcnt)])
        kk = np.arange(len(tidk)) - tstart[tidk]
        grp = kk // P
        lane = kk % P
        mkarr = np.asarray(mks)[sm_k]
        assert (grp < mkarr).all(), "masked group overflow"
        colm = cbase[sm_k] + np.asarray(cks)[sm_k] + grp
        idxlin[sm_core, sm_tl, colm, lane] = sm_relt
        eatt[sm_core, :, (sm_tl * G + colm) * P + lane] = sm_ea
        kmbase = np.zeros(NCH, np.int64)
        acc = 0
        for k in range(NCH):
            kmbase[k] = acc
            acc += mks[k]
        kmg = kmbase[sm_k] + grp
        srclm[sm_core, lane, sm_tl * M + kmg] = sm_p.astype(BF)
        srowm[sm_core, sm_tl, 0, kmg * P + lane] = sm_p.astype(BF)

    eatt = eatt.astype(BF)

    idx16 = np.zeros((N_CORES, P, NT * G * 8), np.int16)
    lin = idxlin.reshape(N_CORES, NT, G * P)
    tgtv = idx16.reshape(N_CORES, P, NT, G * 8)
    for k in range(NCH):
        gk = cks[k] + mks[k]
        if gk == 0:
            continue
        cb = int(cbase[k])
        A_ = lin[:, :, cb * P:(cb + gk) * P]
        W_ = A_.reshape(N_CORES, NT, gk * 8, 16).transpose(0, 3, 1, 2)
        for r in range(8):
            tgtv[:, r * 16:(r + 1) * 16, :, cb * 8:(cb + gk) * 8] = W_

    Xp = np.zeros((NP, DIN), np.float32)
    Xp[:N] = np.asarray(X, np.float32)
    XTT = np.ascontiguousarray(
        Xp.reshape(NT_ALL, P, DIN).transpose(0, 2, 1)).astype(BF)

    Wh = np.asarray(W_heads, np.float32)
    Wo = np.asarray(W_out, np.float32)
    WC = np.ascontiguousarray(Wh.transpose(1, 2, 0).reshape(DIN, DC))
    WAT = np.einsum("hkj,hj->kh", Wh, ah[:, DH:2 * DH])
    WAS = np.einsum("hkj,hj->kh", Wh, ah[:, :DH])
    AE1 = np.ascontiguousarray(AE1m.T)
    AE2 = np.ascontiguousarray(AE2m.T)
    base = np.concatenate(
        [Wo, (Wo @ ao[DOUT:2 * DOUT])[:, None], (Wo @ ao[:DOUT])[:, None]],
        axis=1).astype(np.float32)
    perm = (np.arange(DC) % 8) * DH + np.arange(DC) // 8
    W2X = np.ascontiguousarray(
        base[perm].reshape(4, P, 130).transpose(1, 0, 2).reshape(P, 4 * 130))
    IOTP = np.arange(P, dtype=np.float32)[:, None]
    IOTF = np.tile(np.arange(P, dtype=np.float32)[None, :], (P, 1))
    IDT = np.eye(P, dtype=np.float32)
    ON1 = np.ones((1, P), np.float32)

    in_maps = []
    for cidx in range(N_CORES):
        in_maps.append({
            "xtt": XTT,
            "xto": XTT[cidx * NT:(cidx + 1) * NT],
            "wc": WC.astype(BF), "wat": WAT.astype(BF),
            "was": WAS.astype(BF),
            "ae1": AE1.astype(BF), "ae2": AE2.astype(BF),
            "w2x": W2X.astype(BF),
            "iotp": IOTP.astype(BF), "iotf": IOTF.astype(BF),
            "idt": IDT.astype(BF), "on1": ON1.astype(BF),
            "idx16": idx16[cidx], "srclm": srclm[cidx],
            "srowm": srowm[cidx], "eatt": eatt[cidx],
        })
    meta = dict(N=N, NP=NP, cks=tuple(cks), mks=tuple(mks))
    return in_maps, meta


def make_runner(nc, n_cores=N_CORES):
    """Reusable jitted SPMD executor for a finalized Bass module."""
    import time
    import jax
    from jax.sharding import Mesh, PartitionSpec
    from jax.experimental.shard_map import shard_map
    from concourse import bass2jax
    from concourse.bass2jax import _bass_exec_p, partition_id_tensor

    bass2jax.install_neuronx_cc_hook()
    partition_name = (nc.partition_id_tensor.name
                      if nc.partition_id_tensor else None)
    in_names, out_names, out_avals, zero_outs = [], [], [], []
    for alloc in nc.m.functions[0].allocations:
        if not isinstance(alloc, mybir.MemoryLocationSet):
            continue
        name = alloc.memorylocations[0].name
        if alloc.kind == "ExternalInput":
            if name != partition_name:
                in_names.append(name)
        elif alloc.kind == "ExternalOutput":
            out_names.append(name)
            shape = tuple(alloc.tensor_shape)
            dtype = mybir.dt.np(alloc.dtype)
            out_avals.append(jax.core.ShapedArray(shape, dtype))
            zero_outs.append(np.zeros(shape, dtype))
    n_params = len(in_names)
    all_in_names = list(in_names) + list(out_names)
    if partition_name is not None:
        all_in_names.append(partition_name)

    def _body(*args):
        operands = list(args)
        if partition_name is not None:
            operands.append(partition_id_tensor())
        outs = _bass_exec_p.bind(
            *operands,
            out_avals=tuple(out_avals),
            in_names=tuple(all_in_names),
            out_names=tuple(out_names),
            lowering_input_output_aliases=(),
            sim_require_finite=True,
            sim_require_nnan=True,
            nc=nc,
        )
        return tuple(outs)

    devices = jax.devices()[:n_cores]
    mesh = Mesh(np.asarray(devices), ("core",))
    in_specs = (PartitionSpec("core"),) * (n_params + len(out_names))
    out_specs = (PartitionSpec("core"),) * len(out_names)
    sharded = jax.jit(
        shard_map(_body, mesh=mesh, in_specs=in_specs, out_specs=out_specs,
                  check_rep=False),
        keep_unused=True,
    )

    def run(in_maps, n_iters=0):
        per_core = [[np.asarray(mm[name]) for name in in_names]
                    for mm in in_maps]
        concat_in = [
            np.concatenate([per_core[cc][i] for cc in range(n_cores)], axis=0)
            for i in range(n_params)
        ]
        concat_zeros = [
            np.zeros((n_cores * z.shape[0], *z.shape[1:]), z.dtype)
            for z in zero_outs
        ]
        args = [jax.device_put(a) for a in concat_in]
        args += [jax.device_put(a) for a in concat_zeros]
        out = sharded(*args)
        jax.block_until_ready(out)
        dt = float("nan")
        if n_iters:
            t0 = time.perf_counter()
            for _ in range(n_iters):
                out = sharded(*args)
                jax.block_until_ready(out)
            dt = (time.perf_counter() - t0) / n_iters
        results = [
            {
                name: np.asarray(out[i]).reshape(
                    n_cores, *out_avals[i].shape)[cc]
                for i, name in enumerate(out_names)
            }
            for cc in range(n_cores)
        ]
        return results, dt

    return run


_RUNNER_CACHE = {}


def build_key(meta):
    return (meta["NP"], tuple(meta["cks"]), tuple(meta["mks"]))


def _get_runner(meta):
    key = build_key(meta)
    if key not in _RUNNER_CACHE:
        nc = build_program(*key)
        _RUNNER_CACHE[key] = make_runner(nc, N_CORES)
    return _RUNNER_CACHE[key]


def kernel(X, edge_index, edge_attr, W_heads, a_heads, W_out, a_out):
    in_maps, meta = preprocess(X, edge_index, edge_attr, W_heads, a_heads,
                               W_out, a_out)
    run = _get_runner(meta)
    results, _ = run(in_maps, n_iters=0)
    out = np.concatenate([results[cc]["out"] for cc in range(N_CORES)],
                         axis=0)
    return out[:meta["N"]].astype(np.float32)



# revision 9
# speedup vs baseline: 24.7923x; 1.2264x over previous
"""Trainium2 Bass kernel v4 for the 2-layer multi-head GAT.

Design (8 NeuronCores, SPMD), per core:
  - NO node-table build phase: layer-1 gathers pull 256B rows of X itself
    (bf16, transpose=True -> arrives [DIN, slots], feature-major); h[tgt]
    and s_tgt[tgt] are computed per edge-group on the PE from the gathered
    X columns (h = X^T row @ W), so the old 1280B h-table and its
    build/write phase disappear.
  - Edge slots: per (tile, tgt-chunk) buckets padded to a UNIFORM G_k
    groups of 128 (SPMD requires identical structure on all cores); every
    group is "masked" (maskT/maskv one-hots built on DVE from host lane
    maps); 4 chunks of <=32768 rows (int16 gather indices).
  - Gather calls batched across B=7 tiles per (batch, chunk) for fewer,
    larger SWDGE calls (descriptor-generation on gpsimd is the kernel's
    roofline: ~8ns/row).
  - Scores: s_e (edge_attr @ a_e) folded on the HOST into per-slot tables
    (sea1/sea2, pads = -1e4 so exp->0); s_src via maskv matmuls; s_tgt
    from the gathered X (layer 1) or the gathered G2 row (layer 2).
  - Attention normalization POST-aggregation: pm = sum_e ex_e*h[tgt_e],
    den = sum_e ex_e (same maskT matmuls), h' = pm * (1/den) per lane.
  - Layer 2: x1 -> (transposes) -> W2X -> G2S rows [h2 fp8 | s2tgt bf16 |
    s2src bf16]; AllGather -> G2F; same edge machinery with 1 head;
    fused exact elu + log_softmax.
"""

import numpy as np

import concourse.bass as bass
import concourse.bacc as bacc
import concourse.mybir as mybir
import concourse.tile as tile

F32 = mybir.dt.float32
BF16 = mybir.dt.bfloat16
FP8 = mybir.dt.float8e4
I16 = mybir.dt.int16

N_CORES = 8
P = 128
H = 8
DH = 64
DIN = 128
DC = 512
DOUT = 128
EA = 16
LRELU = 0.01
EPS0 = 1e-30
PAD_SE = -30000.0
CHROWS = 32768

AX = mybir.AxisListType.X
OP = mybir.AluOpType
AF = mybir.ActivationFunctionType


def chunk_layout(NP):
    """(bases, sizes) of tgt chunks of <=32768 rows."""
    cb, cs = [], []
    off = 0
    while off < NP:
        sz = min(CHROWS, NP - off)
        cb.append(off)
        cs.append(sz)
        off += sz
    return tuple(cb), tuple(cs)


def build_program(NT, G_ks, B, maxg_call, CB, CS, tgather=False):
    """One SPMD program for all 8 cores."""
    G_ks = tuple(G_ks)
    NCH = len(G_ks)
    G = sum(G_ks)
    gb = np.cumsum([0] + list(G_ks))        # tile-major group col base
    GOFF = np.cumsum([0] + [B * k for k in G_ks])  # gx col base per chunk
    assert NT % B == 0
    NB = NT // B
    NGB = B * G
    NSH = NT * P
    NP = sum(CS)

    def gxcol(j, c):
        """batch-buffer group column for tile j (in batch), tile-col c."""
        k = int(np.searchsorted(gb, c, side="right")) - 1
        g = c - gb[k]
        return int(GOFF[k]) + j * G_ks[k] + g

    nc = bacc.Bacc("TRN2", target_bir_lowering=False, debug=False,
                   num_devices=N_CORES)

    XTAB = nc.dram_tensor("xtab", [NP, DIN], BF16, kind="ExternalInput")
    XTO = nc.dram_tensor("xto", [NT, DIN, P], BF16, kind="ExternalInput")
    WC = nc.dram_tensor("wc", [DIN, DC], BF16, kind="ExternalInput")
    WAT = nc.dram_tensor("wat", [DIN, H], BF16, kind="ExternalInput")
    WAS = nc.dram_tensor("was", [DIN, H], BF16, kind="ExternalInput")
    W2X = nc.dram_tensor("w2x", [P, 4 * 130], BF16, kind="ExternalInput")
    SEA1 = nc.dram_tensor("sea1", [P, NT * G * H], BF16,
                          kind="ExternalInput")
    SEA2 = nc.dram_tensor("sea2", [P, NT * G], BF16, kind="ExternalInput")
    SROW = nc.dram_tensor("srow", [1, NT * G * P], BF16,
                          kind="ExternalInput")
    SRCL = nc.dram_tensor("srcl", [P, NT * G], BF16, kind="ExternalInput")
    IDX16 = nc.dram_tensor("idx16", [P, NT * G * 8], I16,
                           kind="ExternalInput")
    IOTP = nc.dram_tensor("iotp", [P, 1], BF16, kind="ExternalInput")
    IOTF = nc.dram_tensor("iotf", [P, P], BF16, kind="ExternalInput")
    IDT = nc.dram_tensor("idt", [P, P], BF16, kind="ExternalInput")
    ON1 = nc.dram_tensor("on1", [1, P], BF16, kind="ExternalInput")

    G2S = nc.dram_tensor("g2s", [NSH, 256], FP8)
    G2F = nc.dram_tensor("g2f", [NP, 256], FP8, addr_space="Shared")
    OUT = nc.dram_tensor("out", [NSH, DOUT], F32, kind="ExternalOutput")

    NMC = (G * P + 511) // 512   # maskv build chunks

    with tile.TileContext(nc) as tc, \
         tc.tile_pool(name="const", bufs=1) as cp:
        wc_sb = cp.tile([DIN, DC], BF16, tag="wc")
        nc.sync.dma_start(out=wc_sb[:], in_=WC[:])
        wat_sb = cp.tile([DIN, H], BF16, tag="wat")
        nc.sync.dma_start(out=wat_sb[:], in_=WAT[:])
        was_sb = cp.tile([DIN, H], BF16, tag="was")
        nc.sync.dma_start(out=was_sb[:], in_=WAS[:])
        w2x_sb = cp.tile([P, 4 * 130], BF16, tag="w2x")
        nc.sync.dma_start(out=w2x_sb[:], in_=W2X[:])
        iotp_sb = cp.tile([P, 1], BF16, tag="iotp")
        nc.sync.dma_start(out=iotp_sb[:], in_=IOTP[:])
        iotf_sb = cp.tile([P, P], BF16, tag="iotf")
        nc.sync.dma_start(out=iotf_sb[:], in_=IOTF[:])
        idt_sb = cp.tile([P, P], BF16, tag="idt")
        nc.sync.dma_start(out=idt_sb[:], in_=IDT[:])
        on1_sb = cp.tile([1, P], BF16, tag="on1")
        nc.sync.dma_start(out=on1_sb[:], in_=ON1[:])
        srcl_sb = cp.tile([P, NT * G], BF16, tag="srcl")
        nc.sync.dma_start(out=srcl_sb[:], in_=SRCL[:])
        s2all_sb = cp.tile([P, NT], BF16, tag="s2all")
        neg1_sb = cp.tile([P, 1], F32, tag="neg1")
        nc.vector.memset(neg1_sb[:], -1.0)

        def gather_tile(buf, ixt, table, elem):
            # one call per tgt-chunk; tile-major columns (<=896 idx/call)
            for k in range(NCH):
                ng = G_ks[k]
                c0 = int(gb[k])
                dst = buf[:, c0 * elem:(c0 + ng) * elem] \
                    .rearrange("p (g w) -> p g w", w=elem)
                nc.gpsimd.dma_gather(
                    dst, table[CB[k]:CB[k] + CS[k], :],
                    ixt[:, c0 * 8:(c0 + ng) * 8],
                    ng * P, ng * P, elem)

        def build_masks(mp, srowt, nt, psb):
            maskT = mp.tile([P, G * P], BF16, tag="maskT")
            nc.vector.tensor_tensor(
                out=maskT[:].rearrange("p (g w) -> p g w", w=P),
                in0=srcl_sb[:, nt * G:(nt + 1) * G].unsqueeze(2)
                    .to_broadcast([P, G, P]),
                in1=iotf_sb[:].unsqueeze(1).to_broadcast([P, G, P]),
                op=OP.is_equal)
            maskv = mp.tile([P, G * P], BF16, tag="maskv")
            for c in range(NMC):
                n = min(512, G * P - c * 512)
                pbc = psb.tile([P, 512], F32, tag="pbc")
                nc.tensor.matmul(pbc[:, 0:n], on1_sb[:],
                                 srowt[:, c * 512:c * 512 + n],
                                 start=True, stop=True)
                nc.vector.tensor_tensor(
                    out=maskv[:, c * 512:c * 512 + n], in0=pbc[:, 0:n],
                    in1=iotp_sb[:].to_broadcast([P, n]), op=OP.is_equal)
            return maskT, maskv

        # ================= Phase E1 =================
        with tc.tile_pool(name="e1gx", bufs=3) as gxp, \
             tc.tile_pool(name="e1ix", bufs=3) as ixp, \
             tc.tile_pool(name="e1e", bufs=2) as ep, \
             tc.tile_pool(name="e1m", bufs=2) as mp, \
             tc.tile_pool(name="e1s", bufs=2) as sp, \
             tc.tile_pool(name="e1r", bufs=4) as rp, \
             tc.tile_pool(name="e1x", bufs=2) as xp, \
             tc.tile_pool(name="psh", bufs=2, space="PSUM") as psh, \
             tc.tile_pool(name="psm", bufs=1, space="PSUM") as psm, \
             tc.tile_pool(name="psa", bufs=2, space="PSUM") as psa, \
             tc.tile_pool(name="psb", bufs=1, space="PSUM") as psb, \
             tc.tile_pool(name="pst", bufs=1, space="PSUM") as pst, \
             tc.tile_pool(name="psg", bufs=1, space="PSUM") as psg:
            for nt in range(NT):
                    ixt = ixp.tile([P, G * 8], I16, tag="ixt")
                    nc.sync.dma_start(
                        out=ixt[:], in_=IDX16[:, nt * G * 8:(nt + 1) * G * 8])
                    gxr = gxp.tile([P, G * P], BF16, tag="gxr")
                    gather_tile(gxr, ixt, XTAB, DIN)
                    # transpose gathered [slot, din] groups to [din, slot]
                    # on the PE (4 groups per psum copy)
                    gxt = mp.tile([P, G * P], BF16, tag="gxt")
                    for c0 in range(0, G, 4):
                        cn = min(4, G - c0)
                        ptx = pst.tile([P, DC], BF16, tag="ptx")
                        for cc2 in range(cn):
                            cg = c0 + cc2
                            nc.tensor.transpose(
                                ptx[:, cc2 * P:(cc2 + 1) * P],
                                gxr[:, cg * P:(cg + 1) * P],
                                idt_sb[:])
                        eng = (nc.vector, nc.scalar)[(c0 // 4) % 2]
                        if eng is nc.scalar:
                            nc.scalar.activation(
                                gxt[:, c0 * P:(c0 + cn) * P],
                                ptx[:, 0:cn * P], AF.Copy)
                        else:
                            nc.vector.tensor_copy(
                                out=gxt[:, c0 * P:(c0 + cn) * P],
                                in_=ptx[:, 0:cn * P])
                    sea1t = ep.tile([P, G * H], BF16, tag="sea1t")
                    nc.sync.dma_start(
                        out=sea1t[:],
                        in_=SEA1[:, nt * G * H:(nt + 1) * G * H])
                    srowt = ep.tile([1, G * P], BF16, tag="srowt")
                    nc.sync.dma_start(
                        out=srowt[:],
                        in_=SROW[:, nt * G * P:(nt + 1) * G * P])
                    xtot = ep.tile([DIN, P], BF16, tag="xtot")
                    nc.sync.dma_start(out=xtot[:], in_=XTO[nt])

                    # psa: [s1 0:8 | stp 8:8+G8 | psv ..:8+2G8 | pd ..+8]
                    G8 = G * H
                    pa = psa.tile([P, 16 + 2 * G8], F32, tag="pa")
                    s1p = pa[:, 0:8]
                    stp = pa[:, 8:8 + G8]
                    psv = pa[:, 8 + G8:8 + 2 * G8]
                    pd = pa[:, 8 + 2 * G8:16 + 2 * G8]

                    nc.tensor.matmul(s1p[:], xtot[:], was_sb[:],
                                     start=True, stop=True)
                    s1sb = sp.tile([P, H], BF16, tag="s1sb")
                    nc.scalar.activation(s1sb[:], s1p[:], AF.Copy)

                    maskT, maskv = build_masks(mp, srowt, nt, psb)

                    def glhs(c):
                        return gxt[:, c * P:(c + 1) * P]

                    for c in range(G):
                        nc.tensor.matmul(stp[:, c * 8:(c + 1) * 8], glhs(c),
                                         wat_sb[:], start=True, stop=True)
                        nc.tensor.matmul(psv[:, c * 8:(c + 1) * 8],
                                         maskv[:, c * P:(c + 1) * P],
                                         s1sb[:], start=True, stop=True)
                    scb = sp.tile([P, G8], BF16, tag="scb")
                    nc.vector.tensor_tensor(out=scb[:], in0=stp[:],
                                            in1=sea1t[:], op=OP.add)
                    sc2 = sp.tile([P, G8], BF16, tag="sc2")
                    nc.vector.tensor_tensor(out=sc2[:], in0=psv[:],
                                            in1=scb[:], op=OP.add)
                    lr = sp.tile([P, G8], BF16, tag="lr")
                    nc.vector.scalar_tensor_tensor(
                        out=lr[:], in0=sc2[:], scalar=LRELU, in1=sc2[:],
                        op0=OP.mult, op1=OP.max)
                    ex = sp.tile([P, G8], BF16, tag="ex")
                    nc.scalar.activation(ex[:], lr[:], AF.Exp)

                    for c in range(G):
                        ph = psh.tile([P, DC], F32, tag="ph")
                        nc.tensor.matmul(ph[:], glhs(c), wc_sb[:],
                                         start=True, stop=True)
                        hb = rp.tile([P, DC], BF16, tag="hb")
                        nc.scalar.activation(hb[:], ph[:], AF.Copy)
                        rhs = rp.tile([P, DC], BF16, tag="rhs")
                        nc.vector.tensor_tensor(
                            out=rhs[:].rearrange("p (d h) -> p d h", h=H),
                            in0=hb[:].rearrange("p (d h) -> p d h", h=H),
                            in1=ex[:, c * 8:(c + 1) * 8].unsqueeze(1)
                                .to_broadcast([P, DH, H]),
                            op=OP.mult)
                        if c == 0:
                            pm = psm.tile([P, DC], F32, tag="pm")
                        nc.tensor.matmul(pm[:],
                                         maskT[:, c * P:(c + 1) * P],
                                         rhs[:], start=(c == 0),
                                         stop=(c == G - 1))
                        nc.tensor.matmul(pd[:],
                                         maskT[:, c * P:(c + 1) * P],
                                         ex[:, c * 8:(c + 1) * 8],
                                         start=(c == 0), stop=(c == G - 1))
                    den = sp.tile([P, H], F32, tag="den")
                    nc.vector.tensor_scalar_add(out=den[:], in0=pd[:],
                                                scalar1=EPS0)
                    rcp = sp.tile([P, H], F32, tag="rcp")
                    with nc.allow_low_precision(reason="attn rcp"):
                        nc.vector.reciprocal(out=rcp[:], in_=den[:])
                    x0 = xp.tile([P, DC], BF16, tag="x0")
                    nc.vector.tensor_tensor(
                        out=x0[:].rearrange("p (d h) -> p d h", h=H),
                        in0=pm[:].rearrange("p (d h) -> p d h", h=H),
                        in1=rcp[:].unsqueeze(1).to_broadcast([P, DH, H]),
                        op=OP.mult)
                    # x1 = elu(elu(x0)): reference applies elu in the GAT
                    # layer AND again after the head concat.
                    xa = xp.tile([P, DC], BF16, tag="xa")
                    nc.vector.tensor_scalar_min(out=xa[:], in0=x0[:],
                                                scalar1=0.0)
                    xe = xp.tile([P, DC], BF16, tag="xe")
                    nc.scalar.activation(xe[:], xa[:], AF.Exp)
                    xd = xp.tile([P, DC], BF16, tag="xd")
                    nc.scalar.activation(xd[:], xe[:], AF.Exp,
                                         bias=neg1_sb[:])
                    x1 = xp.tile([P, DC], BF16, tag="x1")
                    nc.vector.scalar_tensor_tensor(
                        out=x1[:], in0=xd[:], scalar=-1.0, in1=x0[:],
                        op0=OP.add, op1=OP.max)
                    pxt = pst.tile([P, DC], BF16, tag="ptx")
                    for c4 in range(4):
                        nc.tensor.transpose(pxt[:, c4 * P:(c4 + 1) * P],
                                            x1[:, c4 * P:(c4 + 1) * P],
                                            idt_sb[:])
                    xts = xp.tile([P, DC], BF16, tag="xts")
                    nc.scalar.activation(xts[:], pxt[:], AF.Copy)
                    pg2 = psg.tile([P, 130], F32, tag="pg2")
                    for c4 in range(4):
                        nc.tensor.matmul(
                            pg2[:], xts[:, c4 * P:(c4 + 1) * P],
                            w2x_sb[:, c4 * 130:(c4 + 1) * 130],
                            start=(c4 == 0), stop=(c4 == 3))
                    g2t = sp.tile([P, 256], FP8, tag="g2t")
                    nc.vector.memset(g2t[:, 132:256], 0.0)
                    nc.vector.tensor_copy(out=g2t[:, 0:DOUT],
                                          in_=pg2[:, 0:DOUT])
                    nc.vector.tensor_copy(
                        out=g2t[:].bitcast(BF16)[:, 64:66],
                        in_=pg2[:, DOUT:DOUT + 2])
                    nc.vector.tensor_copy(out=s2all_sb[:, nt:nt + 1],
                                          in_=pg2[:, 129:130])
                    nc.sync.dma_start(out=G2S[nt * P:(nt + 1) * P, :],
                                      in_=g2t[:])

        tc.strict_bb_all_engine_barrier()

        # ---------------- AllGather G2 shard -> full table -------------
        with tc.tile_critical():
            with nc.semaphore() as cc_sem:
                nc.gpsimd.collective_compute(
                    "AllGather", OP.bypass,
                    replica_groups=[list(range(N_CORES))],
                    ins=[G2S[:]], outs=[G2F[0:NP, :]],
                ).then_inc(cc_sem, 1)
                nc.gpsimd.wait_ge(cc_sem, 1)

        tc.strict_bb_all_engine_barrier()

        # ================= Phase E2 =================
        with tc.tile_pool(name="e2gd", bufs=3) as gdp, \
             tc.tile_pool(name="e2ix", bufs=3) as ixp, \
             tc.tile_pool(name="e2e", bufs=2) as ep, \
             tc.tile_pool(name="e2m", bufs=2) as mp, \
             tc.tile_pool(name="e2s", bufs=2) as sp, \
             tc.tile_pool(name="e2o", bufs=2) as op_, \
             tc.tile_pool(name="psm2", bufs=2, space="PSUM") as psm2, \
             tc.tile_pool(name="psa2", bufs=2, space="PSUM") as psa2, \
             tc.tile_pool(name="psb2", bufs=1, space="PSUM") as psb2:
            for nt in range(NT):
                    ixt = ixp.tile([P, G * 8], I16, tag="ixt")
                    nc.sync.dma_start(
                        out=ixt[:], in_=IDX16[:, nt * G * 8:(nt + 1) * G * 8])
                    gd = gdp.tile([P, G * 256], FP8, tag="gd")
                    gather_tile(gd, ixt, G2F, 256)
                    gdb = gd[:].bitcast(BF16)
                    sea2t = ep.tile([P, G], BF16, tag="sea2t")
                    nc.sync.dma_start(out=sea2t[:],
                                      in_=SEA2[:, nt * G:(nt + 1) * G])
                    srowt = ep.tile([1, G * P], BF16, tag="srowt")
                    nc.sync.dma_start(
                        out=srowt[:],
                        in_=SROW[:, nt * G * P:(nt + 1) * G * P])

                    pa = psa2.tile([P, G + 1], F32, tag="pa2")
                    psv = pa[:, 0:G]
                    pd = pa[:, G:G + 1]

                    maskT, maskv = build_masks(mp, srowt, nt, psb2)

                    for c in range(G):
                        nc.tensor.matmul(psv[:, c:c + 1],
                                         maskv[:, c * P:(c + 1) * P],
                                         s2all_sb[:, nt:nt + 1],
                                         start=True, stop=True)
                    sc = sp.tile([P, G], BF16, tag="sc")
                    nc.vector.tensor_tensor(
                        out=sc[:].unsqueeze(2), in0=psv[:].unsqueeze(2),
                        in1=gdb.rearrange("p (g w) -> p g w", w=128)
                            [:, 0:G, 64:65],
                        op=OP.add)
                    sc2 = sp.tile([P, G], BF16, tag="sc2")
                    nc.vector.tensor_tensor(out=sc2[:], in0=sc[:],
                                            in1=sea2t[:], op=OP.add)
                    lr = sp.tile([P, G], BF16, tag="lr2")
                    nc.vector.scalar_tensor_tensor(
                        out=lr[:], in0=sc2[:], scalar=LRELU, in1=sc2[:],
                        op0=OP.mult, op1=OP.max)
                    ex2 = sp.tile([P, G], BF16, tag="ex2")
                    nc.scalar.activation(ex2[:], lr[:], AF.Exp)

                    rhs2 = sp.tile([P, G * P], BF16, tag="rhs2")
                    nc.vector.tensor_tensor(
                        out=rhs2[:].rearrange("p (g w) -> p g w", w=P),
                        in0=gd[:].rearrange("p (g w) -> p g w", w=256)
                            [:, 0:G, 0:P],
                        in1=ex2[:].unsqueeze(2).to_broadcast([P, G, P]),
                        op=OP.mult)
                    pm2 = psm2.tile([P, DOUT], F32, tag="pm2")
                    for c in range(G):
                        nc.tensor.matmul(pm2[:],
                                         maskT[:, c * P:(c + 1) * P],
                                         rhs2[:, c * P:(c + 1) * P],
                                         start=(c == 0), stop=(c == G - 1))
                        nc.tensor.matmul(pd[:],
                                         maskT[:, c * P:(c + 1) * P],
                                         ex2[:, c:c + 1],
                                         start=(c == 0), stop=(c == G - 1))
                    den2 = sp.tile([P, 1], F32, tag="den2")
                    nc.vector.tensor_scalar_add(out=den2[:], in0=pd[:],
                                                scalar1=EPS0)
                    rcp2 = sp.tile([P, 1], F32, tag="rcp2")
                    with nc.allow_low_precision(reason="attn rcp"):
                        nc.vector.reciprocal(out=rcp2[:], in_=den2[:])
                    h2p = op_.tile([P, DOUT], F32, tag="h2p")
                    nc.vector.tensor_scalar_mul(out=h2p[:], in0=pm2[:],
                                                scalar1=rcp2[:])
                    ha = op_.tile([P, DOUT], F32, tag="ha")
                    nc.vector.tensor_scalar_min(out=ha[:], in0=h2p[:],
                                                scalar1=0.0)
                    he = op_.tile([P, DOUT], F32, tag="he")
                    nc.scalar.activation(he[:], ha[:], AF.Exp)
                    h2e = op_.tile([P, DOUT], F32, tag="h2e")
                    nc.vector.scalar_tensor_tensor(
                        out=h2e[:], in0=he[:], scalar=-1.0, in1=h2p[:],
                        op0=OP.add, op1=OP.max)
                    rmax = op_.tile([P, 1], F32, tag="rmax")
                    nc.vector.tensor_reduce(out=rmax[:], in_=h2e[:],
                                            axis=AX, op=OP.max)
                    z = op_.tile([P, DOUT], F32, tag="z")
                    nc.vector.tensor_scalar_sub(out=z[:], in0=h2e[:],
                                                scalar1=rmax[:])
                    ez = op_.tile([P, DOUT], F32, tag="ez")
                    ssum = op_.tile([P, 1], F32, tag="ssum")
                    nc.scalar.activation(ez[:], z[:], AF.Exp,
                                         accum_out=ssum[:])
                    lnz = op_.tile([P, 1], F32, tag="lnz")
                    nc.scalar.activation(lnz[:], ssum[:], AF.Ln)
                    outt = op_.tile([P, DOUT], F32, tag="outt")
                    nc.vector.tensor_scalar_sub(out=outt[:], in0=z[:],
                                                scalar1=lnz[:])
                    nc.sync.dma_start(out=OUT[nt * P:(nt + 1) * P, :],
                                      in_=outt[:])

    nc.finalize()
    return nc


def pack_idx_groups(vals):
    """vals [ng, 128] int16 -> [128, ng*8] wrapped-16 + replicated x8."""
    ng = vals.shape[0]
    w = vals.reshape(ng, 8, 16).transpose(2, 0, 1).reshape(16, ng * 8)
    return np.tile(w, (8, 1))


def preprocess(X, edge_index, edge_attr, W_heads, a_heads, W_out, a_out,
               B=7, maxg_call=8, tgather=False):
    import ml_dtypes
    BF = ml_dtypes.bfloat16
    N = X.shape[0]
    E = edge_index.shape[1]
    NP = ((N + N_CORES * P - 1) // (N_CORES * P)) * (N_CORES * P)
    NSH = NP // N_CORES
    NT = NSH // P
    NT_ALL = NP // P
    CB, CS = chunk_layout(NP)
    NCH = len(CB)

    src = np.asarray(edge_index[0], dtype=np.int64)
    tgt = np.asarray(edge_index[1], dtype=np.int64)
    gtile = src // P
    chunk = np.searchsorted(np.asarray(CB), tgt, side="right") - 1

    # counts per (gtile, chunk) -> uniform G_k
    cnt = np.zeros((NT_ALL, NCH), np.int64)
    np.add.at(cnt, (gtile, chunk), 1)
    G_ks = tuple(int(x) for x in
                 np.ceil(cnt.max(axis=0) / P).astype(np.int64))
    G = sum(G_ks)
    gb = np.cumsum([0] + list(G_ks))

    order = np.lexsort((tgt, gtile))
    s_s = src[order]
    t_s = tgt[order]
    k_s = chunk[order]
    ea_s = np.asarray(edge_attr, np.float32)[order]
    lane_s = (s_s % P).astype(np.int64)
    core_s = (s_s // NSH).astype(np.int64)
    lt_s = ((s_s % NSH) // P).astype(np.int64)

    # rank within (gtile, chunk)
    keyid = gtile[order] * NCH + k_s
    cnt_flat = np.bincount(keyid, minlength=NT_ALL * NCH)
    kstart = np.concatenate([[0], np.cumsum(cnt_flat)])
    rank = np.arange(E) - kstart[keyid]
    g_s = rank // P
    sp_s = rank % P
    assert (g_s < np.asarray(G_ks)[k_s]).all()
    col_s = gb[k_s] + g_s                      # tile-major group col

    # host-folded edge-attr score projections
    ah = np.asarray(a_heads, np.float32)
    ao = np.asarray(a_out, np.float32)
    AE1m = ah[:, 2 * DH:2 * DH + EA]           # [H, EA]
    ae2 = ao[2 * DOUT:2 * DOUT + EA]           # [EA]
    se1 = ea_s @ AE1m.T                        # [E, H]
    se2 = ea_s @ ae2                           # [E]

    NGRP = NT * G
    sea1 = np.full((N_CORES, P, NGRP * H), PAD_SE, np.float32)
    sea2 = np.full((N_CORES, P, NGRP), PAD_SE, np.float32)
    srow = np.full((N_CORES, 1, NGRP * P), -1.0, np.float32)
    srcl = np.full((N_CORES, P, NGRP), -1.0, np.float32)
    idxm = np.zeros((N_CORES, NGRP, P), np.int16)

    gcol = lt_s * G + col_s
    sea1.reshape(N_CORES, P, NGRP, H)[core_s, sp_s, gcol] = se1
    sea2[core_s, sp_s, gcol] = se2
    srow[core_s, 0, gcol * P + sp_s] = lane_s
    srcl[core_s, sp_s, gcol] = lane_s
    idxm[core_s, gcol, sp_s] = (t_s - np.asarray(CB)[k_s]).astype(np.int16)

    # idx16 tile-major: (tile, group) blocks of 8 columns
    v = idxm.reshape(N_CORES, NGRP, 8, 16) \
        .transpose(0, 3, 1, 2).reshape(N_CORES, 16, NGRP * 8)
    idx16 = np.tile(v, (1, 8, 1))

    Xp = np.zeros((NP, DIN), np.float32)
    Xp[:N] = np.asarray(X, np.float32)
    XTAB = Xp.astype(BF)
    XTO = np.ascontiguousarray(
        Xp.reshape(NT_ALL, P, DIN).transpose(0, 2, 1)).astype(BF)

    Wh = np.asarray(W_heads, np.float32)
    Wo = np.asarray(W_out, np.float32)
    WC = np.ascontiguousarray(Wh.transpose(1, 2, 0).reshape(DIN, DC))
    WAT = np.einsum("hkj,hj->kh", Wh, ah[:, DH:2 * DH])
    WAS = np.einsum("hkj,hj->kh", Wh, ah[:, :DH])
    base = np.concatenate(
        [Wo, (Wo @ ao[DOUT:2 * DOUT])[:, None], (Wo @ ao[:DOUT])[:, None]],
        axis=1).astype(np.float32)
    perm = (np.arange(DC) % 8) * DH + np.arange(DC) // 8
    W2X = np.ascontiguousarray(
        base[perm].reshape(4, P, 130).transpose(1, 0, 2).reshape(P, 4 * 130))
    IOTP = np.arange(P, dtype=np.float32)[:, None]
    IOTF = np.tile(np.arange(P, dtype=np.float32)[None, :], (P, 1))
    IDT = np.eye(P, dtype=np.float32)
    ON1 = np.ones((1, P), np.float32)

    in_maps = []
    for cc in range(N_CORES):
        in_maps.append({
            "xtab": XTAB,
            "xto": XTO[cc * NT:(cc + 1) * NT],
            "wc": WC.astype(BF), "wat": WAT.astype(BF),
            "was": WAS.astype(BF), "w2x": W2X.astype(BF),
            "sea1": sea1[cc].reshape(P, NGRP * H).astype(BF),
            "sea2": sea2[cc].astype(BF),
            "srow": srow[cc].astype(BF),
            "srcl": srcl[cc].astype(BF),
            "idx16": idx16[cc],
            "iotp": IOTP.astype(BF), "iotf": IOTF.astype(BF),
            "idt": IDT.astype(BF), "on1": ON1.astype(BF),
        })
    meta = dict(N=N, NP=NP, NT=NT, G_ks=G_ks, B=B, maxg_call=maxg_call,
                CB=CB, CS=CS, tgather=tgather)
    return in_maps, meta


def build_key(meta):
    return (meta["NT"], tuple(meta["G_ks"]), meta["B"], meta["maxg_call"],
            tuple(meta["CB"]), tuple(meta["CS"]), meta.get("tgather", False))


def make_runner(nc, n_cores=N_CORES):
    """Reusable jitted SPMD executor for a finalized Bass module."""
    import time
    import jax
    from jax.sharding import Mesh, PartitionSpec
    from jax.experimental.shard_map import shard_map
    from concourse import bass2jax
    from concourse.bass2jax import _bass_exec_p, partition_id_tensor

    bass2jax.install_neuronx_cc_hook()
    partition_name = (nc.partition_id_tensor.name
                      if nc.partition_id_tensor else None)
    in_names, out_names, out_avals, zero_outs = [], [], [], []
    for alloc in nc.m.functions[0].allocations:
        if not isinstance(alloc, mybir.MemoryLocationSet):
            continue
        name = alloc.memorylocations[0].name
        if alloc.kind == "ExternalInput":
            if name != partition_name:
                in_names.append(name)
        elif alloc.kind == "ExternalOutput":
            out_names.append(name)
            shape = tuple(alloc.tensor_shape)
            dtype = mybir.dt.np(alloc.dtype)
            out_avals.append(jax.core.ShapedArray(shape, dtype))
            zero_outs.append(np.zeros(shape, dtype))
    n_params = len(in_names)
    all_in_names = list(in_names) + list(out_names)
    if partition_name is not None:
        all_in_names.append(partition_name)

    def _body(*args):
        operands = list(args)
        if partition_name is not None:
            operands.append(partition_id_tensor())
        outs = _bass_exec_p.bind(
            *operands,
            out_avals=tuple(out_avals),
            in_names=tuple(all_in_names),
            out_names=tuple(out_names),
            lowering_input_output_aliases=(),
            sim_require_finite=True,
            sim_require_nnan=True,
            nc=nc,
        )
        return tuple(outs)

    devices = jax.devices()[:n_cores]
    mesh = Mesh(np.asarray(devices), ("core",))
    in_specs = (PartitionSpec("core"),) * (n_params + len(out_names))
    out_specs = (PartitionSpec("core"),) * len(out_names)
    sharded = jax.jit(
        shard_map(_body, mesh=mesh, in_specs=in_specs, out_specs=out_specs,
                  check_rep=False),
        keep_unused=True,
    )

    def run(in_maps, n_iters=0):
        per_core = [[np.asarray(mm[name]) for name in in_names]
                    for mm in in_maps]
        concat_in = [
            np.concatenate([per_core[cc][i] for cc in range(n_cores)],
                           axis=0)
            for i in range(n_params)
        ]
        concat_zeros = [
            np.zeros((n_cores * z.shape[0], *z.shape[1:]), z.dtype)
            for z in zero_outs
        ]
        args = [jax.device_put(a) for a in concat_in]
        args += [jax.device_put(a) for a in concat_zeros]
        out = sharded(*args)
        jax.block_until_ready(out)
        dt = float("nan")
        if n_iters:
            t0 = time.perf_counter()
            for _ in range(n_iters):
                out = sharded(*args)
                jax.block_until_ready(out)
            dt = (time.perf_counter() - t0) / n_iters
        results = [
            {
                name: np.asarray(out[i]).reshape(
                    n_cores, *out_avals[i].shape)[cc]
                for i, name in enumerate(out_names)
            }
            for cc in range(n_cores)
        ]
        return results, dt

    return run


_RUNNER_CACHE = {}


def _get_runner(meta):
    key = build_key(meta)
    if key not in _RUNNER_CACHE:
        nc = build_program(*key)
        _RUNNER_CACHE[key] = make_runner(nc, N_CORES)
    return _RUNNER_CACHE[key]


def kernel(X, edge_index, edge_attr, W_heads, a_heads, W_out, a_out):
    in_maps, meta = preprocess(X, edge_index, edge_attr, W_heads, a_heads,
                               W_out, a_out)
    run = _get_runner(meta)
    results, _ = run(in_maps, n_iters=0)
    out = np.concatenate([results[cc]["out"] for cc in range(N_CORES)],
                         axis=0)
    return out[:meta["N"]].astype(np.float32)
